# revision 24
# baseline (speedup 1.0000x reference)
"""ComPosHGNN Trainium2 kernel: 4-relation heterogeneous GraphConv.

Sharding: each relation's edges are bucketed by DESTINATION range (5000
nodes/core, 8 cores) -- every core computes its own slice of both output
node types, so no collectives are needed.  Host work is layout only
(bucket/sort/pad); all arithmetic (degrees, normalization, projection,
aggregation, relu) runs on device.

Device pipeline (v2 -- bf16 augmented-table design):
  1. deg_out/deg_in from host-padded bf16 weight layouts via reduce_sum,
     then rsqrt (clamped at eps; padding slots have w=0 so exact).
  2. Per SOURCE TABLE (com/pos): ONE shared fp32 read pass builds BOTH
     relations' r_out-scaled bf16 tables (256B rows) in HBM scratch --
     big chunked DMAs, ACT-engine scale+cast (per-partition scale).
  3. Per relation, per 2-dst-tile group and src half: one dma_gather of
     the (host-padded) edge blocks' 256B scaled rows.
  4. Per 128-dst tile: build w-scaled one-hot matrices (bf16) via
     iota-compare; TensorE matmuls scatter-accumulate
     G_t = sum_e w_e*h~[src_e] into fp32 PSUM.
  5. Epilogue: scale rows by rsqrt(deg_in) (ACT), transpose (PE),
     project by W (PE, bf16), add bias, relu (ACT), average the two
     relations per output ntype.
"""
import numpy as np
import ml_dtypes
from contextlib import ExitStack

BF16 = ml_dtypes.bfloat16

N_COM = 40000
N_POS = 40000
D = 128
NCORES = 8
SLICE = N_COM // NCORES          # 5000 dst nodes per core
TILES = 40                       # 39 full 128-row tiles + 1 partial (8 rows)
NT_TAB = 313                     # table tiles (40064 = 313*128 padded rows)
NPAD = NT_TAB * 128
HALF = 20000                     # src-half split for int16 gather indices
EPS = 1e-20
TGRP = 2                         # dst tiles per gather group
NGRP = TILES // TGRP
CH = 8                           # table tiles per scale-pass chunk (312 = 39*8)

# relation -> (src table, dst ntype)
RELS = [
    ("demand", "com", "pos"),
    ("cflow", "com", "com"),
    ("supply", "pos", "com"),
    ("pflow", "pos", "pos"),
]
# src table -> the two relations whose scaled copies share its read pass
AUGS = {"com": ("demand", "cflow"), "pos": ("supply", "pflow")}


def _layout_order():
    """Block-group layout: [tile-group][src-half][tile-within-group]."""
    order = []
    for tg in range(NGRP):
        for h in range(2):
            for t in range(tg * TGRP, (tg + 1) * TGRP):
                order.append((t, h))
    return order


def _wrap_idx16(idx):
    """dma_gather index layout: idx i at [i%16, i//16], tiled x8 (Q7 cores)."""
    assert len(idx) % 16 == 0
    m = idx.astype(np.int16).reshape(-1, 16).T
    return np.tile(m, (8, 1))


def _prep_relation(src, dst, w):
    """Host-side layout for one relation (all cores): gather indices,
    per-block w/dst_local columns, and padded degree arrays."""
    src = np.asarray(src, np.int64)
    dst = np.asarray(dst, np.int64)
    w = np.asarray(w, np.float32)

    # padded-by-src weight array for deg_out (global, shared by all cores)
    counts_s = np.bincount(src, minlength=NPAD)
    P_out = max(8, ((int(counts_s.max()) + 7) // 8) * 8)
    deg_out_pad = np.zeros((NPAD, P_out), np.float32)
    order_s = np.argsort(src, kind="stable")
    ssrc, sw = src[order_s], w[order_s]
    starts = np.zeros(NPAD, np.int64)
    starts[1:] = np.cumsum(counts_s)[:-1]
    deg_out_pad[ssrc, np.arange(len(ssrc)) - starts[ssrc]] = sw
    deg_out_cols = deg_out_pad.reshape(NT_TAB, 128, P_out).transpose(1, 0, 2).reshape(
        128, NT_TAB * P_out).astype(BF16)

    core_of = dst // SLICE
    dloc_all = dst - core_of * SLICE
    tile_all = dloc_all // 128
    half_all = src // HALF
    counts_grid = np.zeros((NCORES, TILES, 2), np.int64)
    for k in range(NCORES):
        m = core_of == k
        np.add.at(counts_grid[k], (tile_all[m], half_all[m]), 1)
    blocks_grid = np.maximum((np.ceil(counts_grid.max(axis=0) / 128)).astype(np.int64), 1)
    NB = int(blocks_grid.sum())

    order = _layout_order()
    # linear sort key per (t, h) following the layout order
    lin_of = np.zeros((TILES, 2), np.int64)
    for i, (t, h) in enumerate(order):
        lin_of[t, h] = i
    # block offset of each (t, h) in the layout
    boff_of = np.zeros((TILES, 2), np.int64)
    off = 0
    for t, h in order:
        boff_of[t, h] = off
        off += int(blocks_grid[t, h])

    # per-core max in-degree (over virtual 5120 rows) for the deg_in pad
    P_in = 8
    percore_masks = []
    for k in range(NCORES):
        m = core_of == k
        percore_masks.append(m)
        cnt_in = np.bincount(dloc_all[m], minlength=5120)
        P_in = max(P_in, ((int(cnt_in.max()) + 7) // 8) * 8)

    per_core = []
    for k in range(NCORES):
        m = percore_masks[k]
        s_k, w_k = src[m], w[m]
        t_k, h_k, dl_k = tile_all[m], half_all[m], dloc_all[m]

        cnt_in = np.bincount(dl_k, minlength=5120)
        deg_in_pad = np.zeros((5120, P_in), np.float32)
        order_d = np.argsort(dl_k, kind="stable")
        sdl, swk = dl_k[order_d], w_k[order_d]
        st = np.zeros(5120, np.int64)
        st[1:] = np.cumsum(cnt_in)[:-1]
        deg_in_pad[sdl, np.arange(len(sdl)) - st[sdl]] = swk
        deg_in_cols = deg_in_pad.reshape(TILES, 128, P_in).transpose(1, 0, 2).reshape(
            128, TILES * P_in).astype(BF16)

        gidx = np.zeros(NB * 128, np.int64)
        wcol = np.zeros(NB * 128, np.float32)
        dcol = np.zeros(NB * 128, np.float32)
        key = lin_of[t_k, h_k]
        eorder = np.argsort(key, kind="stable")
        s_o, w_o, d_o = s_k[eorder], w_k[eorder], dl_k[eorder]
        key_o = key[eorder]
        starts_g = np.searchsorted(key_o, np.arange(TILES * 2))
        ends_g = np.searchsorted(key_o, np.arange(TILES * 2) + 1)
        for t, h in order:
            off = int(boff_of[t, h]) * 128
            a, b = starts_g[lin_of[t, h]], ends_g[lin_of[t, h]]
            n = b - a
            gidx[off:off + n] = s_o[a:b] - h * HALF
            wcol[off:off + n] = w_o[a:b]
            dcol[off:off + n] = d_o[a:b] - t * 128
        per_core.append({
            "gidx": _wrap_idx16(gidx),
            "wcol": wcol.reshape(NB, 128).T.copy(),
            "dcol": dcol.reshape(NB, 128).T.copy(),
            "deg_in": deg_in_cols,
        })
    return per_core, blocks_grid, deg_out_cols, P_out, P_in


def _build_kernel(shapes):
    import concourse.bass as bass  # noqa: F401
    import concourse.tile as tile
    from concourse import bacc, mybir

    f32 = mybir.dt.float32
    bf16 = mybir.dt.bfloat16
    ACT = mybir.ActivationFunctionType
    nc = bacc.Bacc("TRN2", target_bir_lowering=False, debug=False,
                   enable_asserts=False, num_devices=NCORES)

    tabs = {
        "com": nc.dram_tensor("com_emb", [N_COM, D], f32, kind="ExternalInput"),
        "pos": nc.dram_tensor("pos_emb", [N_POS, D], f32, kind="ExternalInput"),
    }
    ins = {}
    for rname, s_t, d_t in RELS:
        sh = shapes[rname]
        NB = int(sh["blocks_grid"].sum())
        ins[rname] = {
            "gidx": nc.dram_tensor(f"{rname}_gidx", [128, NB * 8], mybir.dt.int16,
                                   kind="ExternalInput"),
            "wcol": nc.dram_tensor(f"{rname}_wcol", [128, NB], f32, kind="ExternalInput"),
            "dcol": nc.dram_tensor(f"{rname}_dcol", [128, NB], f32, kind="ExternalInput"),
            "dout": nc.dram_tensor(f"{rname}_degout", [128, NT_TAB * sh["P_out"]], bf16,
                                   kind="ExternalInput"),
            "din": nc.dram_tensor(f"{rname}_degin", [128, TILES * sh["P_in"]], bf16,
                                  kind="ExternalInput"),
            "W": nc.dram_tensor(f"W_{rname}", [D, D], f32, kind="ExternalInput"),
            "b": nc.dram_tensor(f"b_{rname}", [1, D], f32, kind="ExternalInput"),
        }
    # paired scaled tables: row j = [r1*h_j (128 bf16) | r2*h_j (128 bf16)]
    # so chunk stores use 512B descriptors (full DMA rate) while each
    # relation's gather reads its 256B half via elem_step.
    spair = {nt: nc.dram_tensor(f"spair_{nt}", [NPAD, 2 * D], bf16)
             for nt in ("com", "pos")}
    rel_half = {}
    for nt, rels2 in AUGS.items():
        for ri, rel in enumerate(rels2):
            rel_half[rel] = (nt, ri)
    out = nc.dram_tensor("out", [2, SLICE, D], f32, kind="ExternalOutput")

    order = _layout_order()
    # group metadata per relation: (t,h) -> block offset; (tg,h) -> (off, gnb)
    meta = {}
    for rname, s_t, d_t in RELS:
        bg = shapes[rname]["blocks_grid"]
        boff_of = {}
        off = 0
        for t, h in order:
            boff_of[(t, h)] = off
            off += int(bg[t, h])
        grp = {}
        for tg in range(NGRP):
            ts = range(tg * TGRP, (tg + 1) * TGRP)
            for h in range(2):
                goff = boff_of[(ts[0], h)]
                gnb = sum(int(bg[t, h]) for t in ts)
                grp[(tg, h)] = (goff, gnb)
        meta[rname] = (boff_of, grp)
    GB_MAX = max(gnb for rname in meta for (goff, gnb) in meta[rname][1].values())
    DOCHUNK = 64

    with tile.TileContext(nc) as tc:
        with ExitStack() as ctx:
            const_p = ctx.enter_context(tc.tile_pool(name="const", bufs=1))
            deg_p = ctx.enter_context(tc.tile_pool(name="deg", bufs=2))
            stage_p = ctx.enter_context(tc.tile_pool(name="stage", bufs=2))
            idxp = ctx.enter_context(tc.tile_pool(name="idx", bufs=1))
            gp = ctx.enter_context(tc.tile_pool(name="g", bufs=2))
            ohp = ctx.enter_context(tc.tile_pool(name="oh", bufs=4))
            psp = ctx.enter_context(tc.tile_pool(name="ps", bufs=2, space="PSUM"))
            ps2 = ctx.enter_context(tc.tile_pool(name="ps2", bufs=2, space="PSUM"))
            ep = ctx.enter_context(tc.tile_pool(name="ep", bufs=3))
            keep = ctx.enter_context(tc.tile_pool(name="keep", bufs=1))

            # constants: iota row 0..127 (every partition), identity matrix
            iota_i = const_p.tile([128, 128], mybir.dt.int32)
            nc.gpsimd.iota(iota_i[:], pattern=[[1, 128]], base=0, channel_multiplier=0)
            iota_bf = const_p.tile([128, 128], bf16)
            nc.vector.tensor_copy(iota_bf[:], iota_i[:])
            pidx_i = const_p.tile([128, 1], mybir.dt.int32)
            nc.gpsimd.iota(pidx_i[:], pattern=[[1, 1]], base=0, channel_multiplier=1)
            pidx_f = const_p.tile([128, 1], f32)
            nc.vector.tensor_copy(pidx_f[:], pidx_i[:])
            ident_bf = const_p.tile([128, 128], bf16)
            nc.vector.tensor_scalar(ident_bf[:], iota_bf[:], pidx_f[:], None,
                                    op0=mybir.AluOpType.is_equal)

            # --- phase 1: degrees -> rsqrt for all relations ---
            r_in = {}
            r_out = {}
            for rname, s_t, d_t in RELS:
                sh = shapes[rname]
                P_out, P_in = sh["P_out"], sh["P_in"]
                inr = ins[rname]

                ro_t = keep.tile([128, NT_TAB], f32, tag=f"rq_{rname}",
                                 name=f"rq_{rname}")
                dov = inr["dout"].ap().rearrange("p (t q) -> p t q", q=P_out)
                for c0 in range(0, NT_TAB, DOCHUNK):
                    cn = min(DOCHUNK, NT_TAB - c0)
                    do_t = deg_p.tile([128, DOCHUNK * P_out], bf16, tag="dout")
                    dv = do_t[:].rearrange("p (t q) -> p t q", q=P_out)
                    nc.sync.dma_start(dv[:, 0:cn, :], dov[:, c0:c0 + cn, :])
                    nc.vector.reduce_sum(ro_t[:, c0:c0 + cn], dv[:, 0:cn, :],
                                         axis=mybir.AxisListType.X)
                nc.vector.tensor_scalar_max(ro_t[:], ro_t[:], EPS)
                nc.scalar.activation(ro_t[:], ro_t[:], ACT.Sqrt)
                nc.vector.reciprocal(ro_t[:], ro_t[:])
                r_out[rname] = ro_t

                di_t = deg_p.tile([128, TILES * P_in], bf16, tag="din")
                nc.sync.dma_start(di_t[:], inr["din"].ap())
                ri_t = keep.tile([128, TILES], f32, tag=f"rin_{rname}",
                                 name=f"rin_{rname}")
                nc.vector.reduce_sum(ri_t[:],
                                     di_t[:].rearrange("p (t q) -> p t q", q=P_in),
                                     axis=mybir.AxisListType.X)
                nc.vector.tensor_scalar_max(ri_t[:], ri_t[:], EPS)
                nc.scalar.activation(ri_t[:], ri_t[:], ACT.Sqrt)
                nc.vector.reciprocal(ri_t[:], ri_t[:])
                r_in[rname] = ri_t

            # --- phase 2 helper: one shared fp32 read pass per source table
            # builds both relations' r_out-scaled bf16 halves of the paired
            # table ---
            def build_table(nt):
                rels2 = AUGS[nt]
                rawv = tabs[nt].ap()[0:312 * 128, :].rearrange("(j p) d -> p j d", p=128)
                spv = spair[nt].ap()[0:312 * 128, :].rearrange("(j p) d -> p j d", p=128)
                for j0 in range(0, 312, CH):
                    f32st = stage_p.tile([128, CH * D], f32, tag="f32st")
                    fv = f32st[:].rearrange("p (j d) -> p j d", d=D)
                    nc.sync.dma_start(fv[:], rawv[:, j0:j0 + CH, :])
                    bfst = stage_p.tile([128, CH * 2 * D], bf16, tag="bfst")
                    bv = bfst[:].rearrange("p (j d) -> p j d", d=2 * D)
                    for ri, rel in enumerate(rels2):
                        ro_t = r_out[rel]
                        for j in range(CH):
                            nc.scalar.activation(
                                bv[:, j, ri * D:(ri + 1) * D], fv[:, j, :], ACT.Copy,
                                scale=ro_t[:, j0 + j:j0 + j + 1])
                    nc.sync.dma_start(spv[:, j0:j0 + CH, :], bv[:])
                # partial last tile (rows 39936..40000, 64 rows)
                f32p = stage_p.tile([128, D], f32, tag="f32p")
                nc.sync.dma_start(f32p[0:64, :], tabs[nt].ap()[312 * 128:N_COM, :])
                bfp = stage_p.tile([128, 2 * D], bf16, tag="bfp")
                for ri, rel in enumerate(rels2):
                    nc.scalar.activation(bfp[0:64, ri * D:(ri + 1) * D], f32p[0:64, :],
                                         ACT.Copy, scale=r_out[rel][0:64, 312:313])
                nc.sync.dma_start(spair[nt].ap()[312 * 128:N_COM, :], bfp[0:64, :])

            # --- phase 3 helper: per relation edge gather + scatter matmul ---
            acc_out = {
                "com": keep.tile([128, TILES * D], f32, tag="acc_com", name="acc_com"),
                "pos": keep.tile([128, TILES * D], f32, tag="acc_pos", name="acc_pos"),
            }
            first_rel = {"com": True, "pos": True}

            def edge_phase(rname, s_t, d_t):
                sh = shapes[rname]
                bg = sh["blocks_grid"]
                NB = int(bg.sum())
                inr = ins[rname]
                boff_of, grp = meta[rname]

                gidx_t = idxp.tile([128, NB * 8], mybir.dt.int16, tag="gidx")
                nc.sync.dma_start(gidx_t[:], inr["gidx"].ap())
                wcol_t = idxp.tile([128, NB], f32, tag="wcol")
                nc.sync.dma_start(wcol_t[:], inr["wcol"].ap())
                dcol_t = idxp.tile([128, NB], f32, tag="dcol")
                nc.sync.dma_start(dcol_t[:], inr["dcol"].ap())

                W_f = const_p.tile([128, D], f32, tag=f"Wf_{rname}")
                nc.sync.dma_start(W_f[:], inr["W"].ap())
                W_bf = const_p.tile([128, D], bf16, tag=f"Wb_{rname}")
                nc.vector.tensor_copy(W_bf[:], W_f[:])
                b_row = const_p.tile([1, D], f32, tag=f"b_{rname}")
                nc.sync.dma_start(b_row[:], inr["b"].ap())
                b_rep = const_p.tile([128, D], f32, tag=f"brep_{rname}")
                nc.gpsimd.partition_broadcast(b_rep[:], b_row[:])

                nt_h, ri_h = rel_half[rname]
                c0 = ri_h * D
                half_views = [spair[nt_h].ap()[0:HALF, c0:c0 + D],
                              spair[nt_h].ap()[HALF:NPAD, c0:c0 + D]]
                ri_t = r_in[rname]
                acc = acc_out[d_t]

                for tg in range(NGRP):
                    ts = list(range(tg * TGRP, (tg + 1) * TGRP))
                    gviews = {}
                    for h in range(2):
                        goff, gnb = grp[(tg, h)]
                        g = gp.tile([128, GB_MAX * D], bf16, tag=f"g{h}")
                        gv = g[:].rearrange("p (b d) -> p b d", d=D)
                        ni = gnb * 128
                        nc.gpsimd.dma_gather(
                            gv[:, 0:gnb, :], half_views[h],
                            gidx_t[:, goff * 8:(goff + gnb) * 8],
                            num_idxs=ni, num_idxs_reg=ni, elem_size=D,
                            elem_step=2 * D, single_packet=False)
                        gviews[h] = gv
                    for t in ts:
                        ps = psp.tile([128, D], f32, tag="acc")
                        nblocks = [int(bg[t, h]) for h in range(2)]
                        first = True
                        for h in range(2):
                            goff, gnb = grp[(tg, h)]
                            lo = boff_of[(t, h)] - goff
                            gv = gviews[h]
                            for bi in range(nblocks[h]):
                                col = boff_of[(t, h)] + bi
                                slot = lo + bi
                                oh = ohp.tile([128, 128], bf16, tag="oh")
                                nc.vector.tensor_scalar(
                                    oh[:], iota_bf[:],
                                    dcol_t[:, col:col + 1], wcol_t[:, col:col + 1],
                                    op0=mybir.AluOpType.is_equal,
                                    op1=mybir.AluOpType.mult)
                                nc.tensor.matmul(
                                    ps[:], oh[:], gv[:, slot:slot + 1, 0:D],
                                    start=first,
                                    stop=(h == 1 and bi == nblocks[1] - 1))
                                first = False
                        # epilogue: Y = relu((rin*G) @ W + b); acc += 0.5*Y
                        gn = ep.tile([128, D], bf16, tag="gn")
                        nc.scalar.activation(gn[:], ps[:], ACT.Copy,
                                             scale=ri_t[:, t:t + 1])
                        gT_ps = ps2.tile([128, D], bf16, tag="gT")
                        nc.tensor.transpose(gT_ps[:], gn[:], ident_bf[:])
                        gT = ep.tile([128, D], bf16, tag="gTs")
                        nc.scalar.activation(gT[:], gT_ps[:], ACT.Copy)
                        y_ps = ps2.tile([128, D], f32, tag="y")
                        nc.tensor.matmul(y_ps[:], gT[:], W_bf[:], start=True, stop=True)
                        tmp = ep.tile([128, D], f32, tag="tmp")
                        nc.vector.tensor_add(tmp[:], y_ps[:], b_rep[:])
                        if first_rel[d_t]:
                            nc.scalar.activation(acc[:, t * D:(t + 1) * D], tmp[:],
                                                 ACT.Relu, scale=0.5)
                        else:
                            tmp2 = ep.tile([128, D], f32, tag="tmp2")
                            nc.scalar.activation(tmp2[:], tmp[:], ACT.Relu, scale=0.5)
                            nc.vector.tensor_add(
                                acc[:, t * D:(t + 1) * D],
                                acc[:, t * D:(t + 1) * D], tmp2[:])
                first_rel[d_t] = False

            # Emission order interleaves table builds with edge phases so the
            # pos-table scale pass hides under demand's gather DMA.
            build_table("com")
            edge_phase(*RELS[0])   # demand (reads spair_com)
            build_table("pos")
            edge_phase(*RELS[1])   # cflow (reads spair_com)
            edge_phase(*RELS[2])   # supply (reads spair_pos)
            edge_phase(*RELS[3])   # pflow (reads spair_pos)

            # --- phase 4: outputs ---
            for i, ntype in enumerate(("com", "pos")):
                acc = acc_out[ntype]
                accv = acc[:, 0:39 * D].rearrange("p (t d) -> p t d", d=D)
                outv = out.ap()[i, 0:39 * 128, :].rearrange("(t p) d -> p t d", p=128)
                nc.sync.dma_start(outv[:], accv[:])
                nc.sync.dma_start(out.ap()[i, 39 * 128:SLICE, :],
                                  acc[0:8, 39 * D:40 * D])
    nc.compile()
    return nc


LAST_RESULT = None


def kernel(**inputs):
    global LAST_RESULT
    from concourse.bass_utils import run_bass_kernel_spmd

    com_emb = np.asarray(inputs["com_emb"], np.float32)
    pos_emb = np.asarray(inputs["pos_emb"], np.float32)

    shapes, percore_rel = {}, {}
    for rname, s_t, d_t in RELS:
        per_core, blocks_grid, deg_out_cols, P_out, P_in = _prep_relation(
            inputs[f"{rname}_src"], inputs[f"{rname}_dst"], inputs[f"{rname}_w"])
        shapes[rname] = {"blocks_grid": blocks_grid, "P_out": P_out, "P_in": P_in}
        percore_rel[rname] = (per_core, deg_out_cols)

    nc = _build_kernel(shapes)

    in_maps = []
    for k in range(NCORES):
        m = {"com_emb": com_emb, "pos_emb": pos_emb}
        for rname, s_t, d_t in RELS:
            per_core, deg_out_cols = percore_rel[rname]
            pc = per_core[k]
            m[f"{rname}_gidx"] = pc["gidx"]
            m[f"{rname}_wcol"] = pc["wcol"]
            m[f"{rname}_dcol"] = pc["dcol"]
            m[f"{rname}_degout"] = deg_out_cols
            m[f"{rname}_degin"] = pc["deg_in"]
            m[f"W_{rname}"] = np.asarray(inputs[f"W_{rname}"], np.float32)
            m[f"b_{rname}"] = np.asarray(inputs[f"b_{rname}"], np.float32).reshape(1, D)
        in_maps.append(m)

    res = run_bass_kernel_spmd(nc, in_maps, core_ids=list(range(NCORES)))
    LAST_RESULT = res
    out = np.empty((2, N_COM, D), np.float32)
    for k in range(NCORES):
        o = res.results[k]["out"]
        out[0, k * SLICE:(k + 1) * SLICE] = o[0]
        out[1, k * SLICE:(k + 1) * SLICE] = o[1]
    return out


# revision 31
# speedup vs baseline: 6.3066x; 6.3066x over previous
"""ComPosHGNN Trainium2 kernel: 4-relation heterogeneous GraphConv.

Sharding: each relation's edges are bucketed by DESTINATION range (5000
nodes/core, 8 cores) -- every core computes its own slice of both output
node types, so no collectives are needed.  Host work is layout only
(bucket/sort/pad); all arithmetic (degrees, normalization, projection,
aggregation, relu) runs on device.

Device pipeline (v2 -- bf16 augmented-table design):
  1. deg_out/deg_in from host-padded bf16 weight layouts via reduce_sum,
     then rsqrt (clamped at eps; padding slots have w=0 so exact).
  2. Per SOURCE TABLE (com/pos): ONE shared fp32 read pass builds BOTH
     relations' r_out-scaled bf16 tables (256B rows) in HBM scratch --
     big chunked DMAs, ACT-engine scale+cast (per-partition scale).
  3. Per relation, per 2-dst-tile group and src half: one dma_gather of
     the (host-padded) edge blocks' 256B scaled rows.
  4. Per 128-dst tile: build w-scaled one-hot matrices (bf16) via
     iota-compare; TensorE matmuls scatter-accumulate
     G_t = sum_e w_e*h~[src_e] into fp32 PSUM.
  5. Epilogue: scale rows by rsqrt(deg_in) (ACT), transpose (PE),
     project by W (PE, bf16), add bias, relu (ACT), average the two
     relations per output ntype.
"""
import numpy as np
import ml_dtypes
from contextlib import ExitStack

BF16 = ml_dtypes.bfloat16

N_COM = 40000
N_POS = 40000
D = 128
NCORES = 8
SLICE = N_COM // NCORES          # 5000 dst nodes per core
TILES = 40                       # 39 full 128-row tiles + 1 partial (8 rows)
NT_TAB = 313                     # table tiles (40064 = 313*128 padded rows)
NPAD = NT_TAB * 128
HALF = 19968                     # src-half split (128-aligned) for int16 gather
H0T = HALF // 128                # 156 table tiles in half 0
H1R = NPAD - HALF                # 20096 rows in half 1 (157 tiles, last partial)
EPS = 1e-20
TGRP = 2                         # dst tiles per gather group
NGRP = TILES // TGRP
CH = 8                           # table tiles per scale-pass chunk (312 = 39*8)

# relation -> (src table, dst ntype)
RELS = [
    ("demand", "com", "pos"),
    ("cflow", "com", "com"),
    ("supply", "pos", "com"),
    ("pflow", "pos", "pos"),
]
# src table -> the two relations whose scaled copies share its read pass
AUGS = {"com": ("demand", "cflow"), "pos": ("supply", "pflow")}


def _layout_order():
    """Block-group layout: [tile-group][src-half][tile-within-group]."""
    order = []
    for tg in range(NGRP):
        for h in range(2):
            for t in range(tg * TGRP, (tg + 1) * TGRP):
                order.append((t, h))
    return order


def _wrap_idx16(idx):
    """dma_gather index layout: idx i at [i%16, i//16], tiled x8 (Q7 cores)."""
    assert len(idx) % 16 == 0
    m = idx.astype(np.int16).reshape(-1, 16).T
    return np.tile(m, (8, 1))


def _prep_relation(src, dst, w):
    """Host-side layout for one relation (all cores): gather indices,
    per-block w/dst_local columns, and padded degree arrays."""
    src = np.asarray(src, np.int64)
    dst = np.asarray(dst, np.int64)
    w = np.asarray(w, np.float32)

    # padded-by-src weight array for deg_out (global, shared by all cores)
    counts_s = np.bincount(src, minlength=NPAD)
    P_out = max(8, ((int(counts_s.max()) + 7) // 8) * 8)
    deg_out_pad = np.zeros((NPAD, P_out), np.float32)
    order_s = np.argsort(src, kind="stable")
    ssrc, sw = src[order_s], w[order_s]
    starts = np.zeros(NPAD, np.int64)
    starts[1:] = np.cumsum(counts_s)[:-1]
    deg_out_pad[ssrc, np.arange(len(ssrc)) - starts[ssrc]] = sw
    deg_out_cols = deg_out_pad.reshape(NT_TAB, 128, P_out).transpose(1, 0, 2).reshape(
        128, NT_TAB * P_out).astype(BF16)

    core_of = dst // SLICE
    dloc_all = dst - core_of * SLICE
    tile_all = dloc_all // 128
    half_all = (src >= HALF).astype(np.int64)
    counts_grid = np.zeros((NCORES, TILES, 2), np.int64)
    for k in range(NCORES):
        m = core_of == k
        np.add.at(counts_grid[k], (tile_all[m], half_all[m]), 1)
    blocks_grid = np.maximum((np.ceil(counts_grid.max(axis=0) / 128)).astype(np.int64), 1)
    NB = int(blocks_grid.sum())

    order = _layout_order()
    # linear sort key per (t, h) following the layout order
    lin_of = np.zeros((TILES, 2), np.int64)
    for i, (t, h) in enumerate(order):
        lin_of[t, h] = i
    # block offset of each (t, h) in the layout
    boff_of = np.zeros((TILES, 2), np.int64)
    off = 0
    for t, h in order:
        boff_of[t, h] = off
        off += int(blocks_grid[t, h])

    # per-core max in-degree (over virtual 5120 rows) for the deg_in pad
    P_in = 8
    percore_masks = []
    for k in range(NCORES):
        m = core_of == k
        percore_masks.append(m)
        cnt_in = np.bincount(dloc_all[m], minlength=5120)
        P_in = max(P_in, ((int(cnt_in.max()) + 7) // 8) * 8)

    per_core = []
    for k in range(NCORES):
        m = percore_masks[k]
        s_k, w_k = src[m], w[m]
        t_k, h_k, dl_k = tile_all[m], half_all[m], dloc_all[m]

        cnt_in = np.bincount(dl_k, minlength=5120)
        deg_in_pad = np.zeros((5120, P_in), np.float32)
        order_d = np.argsort(dl_k, kind="stable")
        sdl, swk = dl_k[order_d], w_k[order_d]
        st = np.zeros(5120, np.int64)
        st[1:] = np.cumsum(cnt_in)[:-1]
        deg_in_pad[sdl, np.arange(len(sdl)) - st[sdl]] = swk
        deg_in_cols = deg_in_pad.reshape(TILES, 128, P_in).transpose(1, 0, 2).reshape(
            128, TILES * P_in).astype(BF16)

        gidx = np.zeros(NB * 128, np.int64)
        wcol = np.zeros(NB * 128, np.float32)
        dcol = np.zeros(NB * 128, np.float32)
        key = lin_of[t_k, h_k]
        eorder = np.argsort(key, kind="stable")
        s_o, w_o, d_o = s_k[eorder], w_k[eorder], dl_k[eorder]
        key_o = key[eorder]
        starts_g = np.searchsorted(key_o, np.arange(TILES * 2))
        ends_g = np.searchsorted(key_o, np.arange(TILES * 2) + 1)
        for t, h in order:
            off = int(boff_of[t, h]) * 128
            a, b = starts_g[lin_of[t, h]], ends_g[lin_of[t, h]]
            n = b - a
            gidx[off:off + n] = s_o[a:b] - h * HALF
            wcol[off:off + n] = w_o[a:b]
            dcol[off:off + n] = d_o[a:b] - t * 128
        per_core.append({
            "gidx": _wrap_idx16(gidx),
            "wcol": wcol.reshape(NB, 128).T.copy(),
            "dcol": dcol.reshape(NB, 128).T.copy(),
            "deg_in": deg_in_cols,
        })
    return per_core, blocks_grid, deg_out_cols, P_out, P_in


def _build_kernel(shapes):
    import concourse.bass as bass  # noqa: F401
    import concourse.tile as tile
    from concourse import bacc, mybir

    f32 = mybir.dt.float32
    bf16 = mybir.dt.bfloat16
    ACT = mybir.ActivationFunctionType
    nc = bacc.Bacc("TRN2", target_bir_lowering=False, debug=False,
                   enable_asserts=False, num_devices=NCORES)

    tabs = {
        "com": nc.dram_tensor("com_emb", [N_COM, D], f32, kind="ExternalInput"),
        "pos": nc.dram_tensor("pos_emb", [N_POS, D], f32, kind="ExternalInput"),
    }
    ins = {}
    for rname, s_t, d_t in RELS:
        sh = shapes[rname]
        NB = int(sh["blocks_grid"].sum())
        ins[rname] = {
            "gidx": nc.dram_tensor(f"{rname}_gidx", [128, NB * 8], mybir.dt.int16,
                                   kind="ExternalInput"),
            "wcol": nc.dram_tensor(f"{rname}_wcol", [128, NB], f32, kind="ExternalInput"),
            "dcol": nc.dram_tensor(f"{rname}_dcol", [128, NB], f32, kind="ExternalInput"),
            "dout": nc.dram_tensor(f"{rname}_degout", [128, NT_TAB * sh["P_out"]], bf16,
                                   kind="ExternalInput"),
            "din": nc.dram_tensor(f"{rname}_degin", [128, TILES * sh["P_in"]], bf16,
                                  kind="ExternalInput"),
            "W": nc.dram_tensor(f"W_{rname}", [D, D], f32, kind="ExternalInput"),
            "b": nc.dram_tensor(f"b_{rname}", [1, D], f32, kind="ExternalInput"),
        }
    # paired scaled tables: row j = [r1*h_j (128 bf16) | r2*h_j (128 bf16)]
    # so chunk stores use 512B descriptors (full DMA rate) while each
    # relation's gather reads its 256B half via elem_step. Each table is
    # split into the two src-half tensors so h0 gathers only depend on the
    # first half of the build pass.
    spair = {nt: (nc.dram_tensor(f"spair_{nt}_h0", [HALF, 2 * D], bf16),
                  nc.dram_tensor(f"spair_{nt}_h1", [H1R, 2 * D], bf16))
             for nt in ("com", "pos")}
    rel_half = {}
    for nt, rels2 in AUGS.items():
        for ri, rel in enumerate(rels2):
            rel_half[rel] = (nt, ri)
    out = nc.dram_tensor("out", [2, SLICE, D], f32, kind="ExternalOutput")

    order = _layout_order()
    # group metadata per relation: (t,h) -> block offset; (tg,h) -> (off, gnb)
    meta = {}
    for rname, s_t, d_t in RELS:
        bg = shapes[rname]["blocks_grid"]
        boff_of = {}
        off = 0
        for t, h in order:
            boff_of[(t, h)] = off
            off += int(bg[t, h])
        grp = {}
        for tg in range(NGRP):
            ts = range(tg * TGRP, (tg + 1) * TGRP)
            for h in range(2):
                goff = boff_of[(ts[0], h)]
                gnb = sum(int(bg[t, h]) for t in ts)
                grp[(tg, h)] = (goff, gnb)
        meta[rname] = (boff_of, grp)
    GB_MAX = max(gnb for rname in meta for (goff, gnb) in meta[rname][1].values())
    DOCHUNK = 64

    with tile.TileContext(nc) as tc:
        with ExitStack() as ctx:
            const_p = ctx.enter_context(tc.tile_pool(name="const", bufs=1))
            deg_p = ctx.enter_context(tc.tile_pool(name="deg", bufs=2))
            stage_p = ctx.enter_context(tc.tile_pool(name="stage", bufs=2))
            idxp = ctx.enter_context(tc.tile_pool(name="idx", bufs=1))
            gp = ctx.enter_context(tc.tile_pool(name="g", bufs=2))
            ohp = ctx.enter_context(tc.tile_pool(name="oh", bufs=4))
            psp = ctx.enter_context(tc.tile_pool(name="ps", bufs=2, space="PSUM"))
            ps2 = ctx.enter_context(tc.tile_pool(name="ps2", bufs=2, space="PSUM"))
            ep = ctx.enter_context(tc.tile_pool(name="ep", bufs=3))
            keep = ctx.enter_context(tc.tile_pool(name="keep", bufs=1))

            # constants: iota row 0..127 (every partition), identity matrix
            iota_i = const_p.tile([128, 128], mybir.dt.int32)
            nc.gpsimd.iota(iota_i[:], pattern=[[1, 128]], base=0, channel_multiplier=0)
            iota_bf = const_p.tile([128, 128], bf16)
            nc.vector.tensor_copy(iota_bf[:], iota_i[:])
            pidx_i = const_p.tile([128, 1], mybir.dt.int32)
            nc.gpsimd.iota(pidx_i[:], pattern=[[1, 1]], base=0, channel_multiplier=1)
            pidx_f = const_p.tile([128, 1], f32)
            nc.vector.tensor_copy(pidx_f[:], pidx_i[:])
            ident_bf = const_p.tile([128, 128], bf16)
            nc.vector.tensor_scalar(ident_bf[:], iota_bf[:], pidx_f[:], None,
                                    op0=mybir.AluOpType.is_equal)

            # --- phase 1 helpers: degrees -> rsqrt ---
            r_in = {}
            r_out = {}

            def deg_out_phase(rname):
                sh = shapes[rname]
                P_out = sh["P_out"]
                inr = ins[rname]
                ro_t = keep.tile([128, NT_TAB], f32, tag=f"rq_{rname}",
                                 name=f"rq_{rname}")
                dov = inr["dout"].ap().rearrange("p (t q) -> p t q", q=P_out)
                for c0 in range(0, NT_TAB, DOCHUNK):
                    cn = min(DOCHUNK, NT_TAB - c0)
                    do_t = deg_p.tile([128, DOCHUNK * P_out], bf16, tag="dout")
                    dv = do_t[:].rearrange("p (t q) -> p t q", q=P_out)
                    nc.sync.dma_start(dv[:, 0:cn, :], dov[:, c0:c0 + cn, :])
                    nc.vector.reduce_sum(ro_t[:, c0:c0 + cn], dv[:, 0:cn, :],
                                         axis=mybir.AxisListType.X)
                nc.vector.tensor_scalar_max(ro_t[:], ro_t[:], EPS)
                nc.scalar.activation(ro_t[:], ro_t[:], ACT.Sqrt)
                nc.vector.reciprocal(ro_t[:], ro_t[:])
                r_out[rname] = ro_t

            def deg_in_phase(rname):
                sh = shapes[rname]
                P_in = sh["P_in"]
                inr = ins[rname]
                di_t = deg_p.tile([128, TILES * P_in], bf16, tag="din")
                nc.sync.dma_start(di_t[:], inr["din"].ap())
                ri_t = keep.tile([128, TILES], f32, tag=f"rin_{rname}",
                                 name=f"rin_{rname}")
                nc.vector.reduce_sum(ri_t[:],
                                     di_t[:].rearrange("p (t q) -> p t q", q=P_in),
                                     axis=mybir.AxisListType.X)
                nc.vector.tensor_scalar_max(ri_t[:], ri_t[:], EPS)
                nc.scalar.activation(ri_t[:], ri_t[:], ACT.Sqrt)
                nc.vector.reciprocal(ri_t[:], ri_t[:])
                r_in[rname] = ri_t

            # --- phase 2 helper: one shared fp32 read pass per source table
            # builds both relations' r_out-scaled bf16 halves of the paired
            # split tables. Returns a list of per-chunk emitters so callers
            # can interleave the build with edge-phase emission. The two
            # relations' scale-casts split across ACT and DVE. ---
            def build_chunks(nt):
                rels2 = AUGS[nt]

                def mk_chunk(hh, jloc, g0, jn):
                    def emit():
                        rawv = tabs[nt].ap()[g0 * 128:(g0 + jn) * 128, :].rearrange(
                            "(j p) d -> p j d", p=128)
                        spv = spair[nt][hh].ap()[jloc * 128:(jloc + jn) * 128, :
                                                 ].rearrange("(j p) d -> p j d", p=128)
                        f32st = stage_p.tile([128, CH * D], f32, tag="f32st")
                        fv = f32st[:].rearrange("p (j d) -> p j d", d=D)
                        nc.sync.dma_start(fv[:, 0:jn, :], rawv[:])
                        bfst = stage_p.tile([128, CH * 2 * D], bf16, tag="bfst")
                        bv = bfst[:].rearrange("p (j d) -> p j d", d=2 * D)
                        for j in range(jn):
                            nc.scalar.activation(
                                bv[:, j, 0:D], fv[:, j, :], ACT.Copy,
                                scale=r_out[rels2[0]][:, g0 + j:g0 + j + 1])
                            nc.vector.tensor_scalar_mul(
                                bv[:, j, D:2 * D], fv[:, j, :],
                                r_out[rels2[1]][:, g0 + j:g0 + j + 1])
                        nc.sync.dma_start(spv[:], bv[:, 0:jn, :])
                    return emit

                chunks = []
                for hh, (t0, ntile) in enumerate(((0, H0T), (H0T, 312 - H0T))):
                    for j0 in range(0, ntile, CH):
                        jn = min(CH, ntile - j0)
                        chunks.append(mk_chunk(hh, j0, t0 + j0, jn))

                def emit_partial():
                    # last tile (rows 39936..40000, 64 rows -> h1 local rows)
                    f32p = stage_p.tile([128, D], f32, tag="f32p")
                    nc.sync.dma_start(f32p[0:64, :], tabs[nt].ap()[312 * 128:N_COM, :])
                    bfp = stage_p.tile([128, 2 * D], bf16, tag="bfp")
                    nc.scalar.activation(bfp[0:64, 0:D], f32p[0:64, :],
                                         ACT.Copy, scale=r_out[rels2[0]][0:64, 312:313])
                    nc.vector.tensor_scalar_mul(bfp[0:64, D:2 * D], f32p[0:64, :],
                                                r_out[rels2[1]][0:64, 312:313])
                    lo = 312 * 128 - HALF
                    nc.sync.dma_start(spair[nt][1].ap()[lo:lo + 64, :], bfp[0:64, :])
                chunks.append(emit_partial)
                return chunks

            # --- phase 3 helper: per relation edge gather + scatter matmul ---
            acc_out = {
                "com": keep.tile([128, TILES * D], f32, tag="acc_com", name="acc_com"),
                "pos": keep.tile([128, TILES * D], f32, tag="acc_pos", name="acc_pos"),
            }
            first_rel = {"com": True, "pos": True}

            def edge_phase(rname, s_t, d_t, interleave=None):
                sh = shapes[rname]
                bg = sh["blocks_grid"]
                NB = int(bg.sum())
                inr = ins[rname]
                boff_of, grp = meta[rname]

                gidx_t = idxp.tile([128, NB * 8], mybir.dt.int16, tag="gidx")
                nc.sync.dma_start(gidx_t[:], inr["gidx"].ap())
                wcol_t = idxp.tile([128, NB], f32, tag="wcol")
                nc.sync.dma_start(wcol_t[:], inr["wcol"].ap())
                dcol_t = idxp.tile([128, NB], f32, tag="dcol")
                nc.sync.dma_start(dcol_t[:], inr["dcol"].ap())

                W_f = const_p.tile([128, D], f32, tag=f"Wf_{rname}")
                nc.sync.dma_start(W_f[:], inr["W"].ap())
                W_bf = const_p.tile([128, D], bf16, tag=f"Wb_{rname}")
                nc.vector.tensor_copy(W_bf[:], W_f[:])
                b_row = const_p.tile([1, D], f32, tag=f"b_{rname}")
                nc.sync.dma_start(b_row[:], inr["b"].ap())
                b_rep = const_p.tile([128, D], f32, tag=f"brep_{rname}")
                nc.gpsimd.partition_broadcast(b_rep[:], b_row[:])

                nt_h, ri_h = rel_half[rname]
                c0 = ri_h * D
                half_views = [spair[nt_h][0].ap()[0:HALF, c0:c0 + D],
                              spair[nt_h][1].ap()[0:H1R, c0:c0 + D]]
                ri_t = r_in[rname]
                acc = acc_out[d_t]

                for tg in range(NGRP):
                    if interleave:
                        for _ in range(3):
                            if interleave:
                                interleave.pop(0)()
                    ts = list(range(tg * TGRP, (tg + 1) * TGRP))
                    gviews = {}
                    for h in range(2):
                        goff, gnb = grp[(tg, h)]
                        g = gp.tile([128, GB_MAX * D], bf16, tag=f"g{h}")
                        gv = g[:].rearrange("p (b d) -> p b d", d=D)
                        ni = gnb * 128
                        nc.gpsimd.dma_gather(
                            gv[:, 0:gnb, :], half_views[h],
                            gidx_t[:, goff * 8:(goff + gnb) * 8],
                            num_idxs=ni, num_idxs_reg=ni, elem_size=D,
                            elem_step=2 * D, single_packet=False)
                        gviews[h] = gv
                    for t in ts:
                        ps = psp.tile([128, D], f32, tag="acc")
                        nblocks = [int(bg[t, h]) for h in range(2)]
                        first = True
                        for h in range(2):
                            goff, gnb = grp[(tg, h)]
                            lo = boff_of[(t, h)] - goff
                            gv = gviews[h]
                            for bi in range(nblocks[h]):
                                col = boff_of[(t, h)] + bi
                                slot = lo + bi
                                oh = ohp.tile([128, 128], bf16, tag="oh")
                                nc.vector.tensor_scalar(
                                    oh[:], iota_bf[:],
                                    dcol_t[:, col:col + 1], wcol_t[:, col:col + 1],
                                    op0=mybir.AluOpType.is_equal,
                                    op1=mybir.AluOpType.mult)
                                nc.tensor.matmul(
                                    ps[:], oh[:], gv[:, slot:slot + 1, 0:D],
                                    start=first,
                                    stop=(h == 1 and bi == nblocks[1] - 1))
                                first = False
                        # epilogue: Y = relu((rin*G) @ W + b); acc += 0.5*Y
                        gn = ep.tile([128, D], bf16, tag="gn")
                        nc.scalar.activation(gn[:], ps[:], ACT.Copy,
                                             scale=ri_t[:, t:t + 1])
                        gT_ps = ps2.tile([128, D], bf16, tag="gT")
                        nc.tensor.transpose(gT_ps[:], gn[:], ident_bf[:])
                        gT = ep.tile([128, D], bf16, tag="gTs")
                        nc.scalar.activation(gT[:], gT_ps[:], ACT.Copy)
                        y_ps = ps2.tile([128, D], f32, tag="y")
                        nc.tensor.matmul(y_ps[:], gT[:], W_bf[:], start=True, stop=True)
                        tmp = ep.tile([128, D], f32, tag="tmp")
                        nc.vector.tensor_add(tmp[:], y_ps[:], b_rep[:])
                        if first_rel[d_t]:
                            nc.scalar.activation(acc[:, t * D:(t + 1) * D], tmp[:],
                                                 ACT.Relu, scale=0.5)
                        else:
                            tmp2 = ep.tile([128, D], f32, tag="tmp2")
                            nc.scalar.activation(tmp2[:], tmp[:], ACT.Relu, scale=0.5)
                            nc.vector.tensor_add(
                                acc[:, t * D:(t + 1) * D],
                                acc[:, t * D:(t + 1) * D], tmp2[:])
                first_rel[d_t] = False

            # Emission order: only demand+cflow deg_out gates the com build;
            # the pos-table build chunks interleave into demand's edge loop
            # so its scale pass hides under demand's gather DMA.
            deg_out_phase("demand")
            deg_out_phase("cflow")
            for emit in build_chunks("com"):
                emit()
            deg_out_phase("supply")
            deg_out_phase("pflow")
            for r, _, _ in RELS:
                deg_in_phase(r)
            pos_chunks = build_chunks("pos")
            edge_phase(*RELS[0], interleave=pos_chunks)   # demand (spair_com)
            for emit in pos_chunks:   # any leftovers
                emit()
            pos_chunks.clear()
            edge_phase(*RELS[1])   # cflow (reads spair_com)
            edge_phase(*RELS[2])   # supply (reads spair_pos)
            edge_phase(*RELS[3])   # pflow (reads spair_pos)

            # --- phase 4: outputs ---
            for i, ntype in enumerate(("com", "pos")):
                acc = acc_out[ntype]
                accv = acc[:, 0:39 * D].rearrange("p (t d) -> p t d", d=D)
                outv = out.ap()[i, 0:39 * 128, :].rearrange("(t p) d -> p t d", p=128)
                nc.sync.dma_start(outv[:], accv[:])
                nc.sync.dma_start(out.ap()[i, 39 * 128:SLICE, :],
                                  acc[0:8, 39 * D:40 * D])
    nc.compile()
    return nc


LAST_RESULT = None


def kernel(**inputs):
    global LAST_RESULT
    from concourse.bass_utils import run_bass_kernel_spmd

    com_emb = np.asarray(inputs["com_emb"], np.float32)
    pos_emb = np.asarray(inputs["pos_emb"], np.float32)

    shapes, percore_rel = {}, {}
    for rname, s_t, d_t in RELS:
        per_core, blocks_grid, deg_out_cols, P_out, P_in = _prep_relation(
            inputs[f"{rname}_src"], inputs[f"{rname}_dst"], inputs[f"{rname}_w"])
        shapes[rname] = {"blocks_grid": blocks_grid, "P_out": P_out, "P_in": P_in}
        percore_rel[rname] = (per_core, deg_out_cols)

    nc = _build_kernel(shapes)

    in_maps = []
    for k in range(NCORES):
        m = {"com_emb": com_emb, "pos_emb": pos_emb}
        for rname, s_t, d_t in RELS:
            per_core, deg_out_cols = percore_rel[rname]
            pc = per_core[k]
            m[f"{rname}_gidx"] = pc["gidx"]
            m[f"{rname}_wcol"] = pc["wcol"]
            m[f"{rname}_dcol"] = pc["dcol"]
            m[f"{rname}_degout"] = deg_out_cols
            m[f"{rname}_degin"] = pc["deg_in"]
            m[f"W_{rname}"] = np.asarray(inputs[f"W_{rname}"], np.float32)
            m[f"b_{rname}"] = np.asarray(inputs[f"b_{rname}"], np.float32).reshape(1, D)
        in_maps.append(m)

    res = run_bass_kernel_spmd(nc, in_maps, core_ids=list(range(NCORES)))
    LAST_RESULT = res
    out = np.empty((2, N_COM, D), np.float32)
    for k in range(NCORES):
        o = res.results[k]["out"]
        out[0, k * SLICE:(k + 1) * SLICE] = o[0]
        out[1, k * SLICE:(k + 1) * SLICE] = o[1]
    return out


# revision 34
# speedup vs baseline: 6.5448x; 1.0378x over previous
"""ComPosHGNN Trainium2 kernel: 4-relation heterogeneous GraphConv.

Sharding: each relation's edges are bucketed by DESTINATION range (5000
nodes/core, 8 cores) -- every core computes its own slice of both output
node types, so no collectives are needed.  Host work is layout only
(bucket/sort/pad); all arithmetic (degrees, normalization, projection,
aggregation, relu) runs on device.

Device pipeline (v2 -- bf16 augmented-table design):
  1. deg_out/deg_in from host-padded bf16 weight layouts via reduce_sum,
     then rsqrt (clamped at eps; padding slots have w=0 so exact).
  2. Per SOURCE TABLE (com/pos): ONE shared fp32 read pass builds BOTH
     relations' r_out-scaled bf16 tables (256B rows) in HBM scratch --
     big chunked DMAs, ACT-engine scale+cast (per-partition scale).
  3. Per relation, per 2-dst-tile group and src half: one dma_gather of
     the (host-padded) edge blocks' 256B scaled rows.
  4. Per 128-dst tile: build w-scaled one-hot matrices (bf16) via
     iota-compare; TensorE matmuls scatter-accumulate
     G_t = sum_e w_e*h~[src_e] into fp32 PSUM.
  5. Epilogue: scale rows by rsqrt(deg_in) (ACT), transpose (PE),
     project by W (PE, bf16), add bias, relu (ACT), average the two
     relations per output ntype.
"""
import numpy as np
import ml_dtypes
from contextlib import ExitStack

BF16 = ml_dtypes.bfloat16

N_COM = 40000
N_POS = 40000
D = 128
NCORES = 8
SLICE = N_COM // NCORES          # 5000 dst nodes per core
TILES = 40                       # 39 full 128-row tiles + 1 partial (8 rows)
NT_TAB = 313                     # table tiles (40064 = 313*128 padded rows)
NPAD = NT_TAB * 128
HALF = 19968                     # src-half split (128-aligned) for int16 gather
H0T = HALF // 128                # 156 table tiles in half 0
H1R = NPAD - HALF                # 20096 rows in half 1 (157 tiles, last partial)
EPS = 1e-20
TGRP = 2                         # dst tiles per gather group
NGRP = TILES // TGRP
CH = 8                           # table tiles per scale-pass chunk (312 = 39*8)

# relation -> (src table, dst ntype)
RELS = [
    ("demand", "com", "pos"),
    ("cflow", "com", "com"),
    ("supply", "pos", "com"),
    ("pflow", "pos", "pos"),
]
# src table -> the two relations whose scaled copies share its read pass
AUGS = {"com": ("demand", "cflow"), "pos": ("supply", "pflow")}


def _layout_order():
    """Block-group layout: [tile-group][src-half][tile-within-group]."""
    order = []
    for tg in range(NGRP):
        for h in range(2):
            for t in range(tg * TGRP, (tg + 1) * TGRP):
                order.append((t, h))
    return order


def _wrap_idx16(idx):
    """dma_gather index layout: idx i at [i%16, i//16], tiled x8 (Q7 cores)."""
    assert len(idx) % 16 == 0
    m = idx.astype(np.int16).reshape(-1, 16).T
    return np.tile(m, (8, 1))


def _prep_relation(src, dst, w):
    """Host-side layout for one relation (all cores): gather indices,
    per-block w/dst_local columns, and padded degree arrays."""
    src = np.asarray(src, np.int64)
    dst = np.asarray(dst, np.int64)
    w = np.asarray(w, np.float32)

    # padded-by-src weight array for deg_out (global, shared by all cores)
    counts_s = np.bincount(src, minlength=NPAD)
    P_out = max(8, ((int(counts_s.max()) + 7) // 8) * 8)
    deg_out_pad = np.zeros((NPAD, P_out), np.float32)
    order_s = np.argsort(src, kind="stable")
    ssrc, sw = src[order_s], w[order_s]
    starts = np.zeros(NPAD, np.int64)
    starts[1:] = np.cumsum(counts_s)[:-1]
    deg_out_pad[ssrc, np.arange(len(ssrc)) - starts[ssrc]] = sw
    deg_out_cols = deg_out_pad.reshape(NT_TAB, 128, P_out).transpose(1, 0, 2).reshape(
        128, NT_TAB * P_out).astype(BF16)

    core_of = dst // SLICE
    dloc_all = dst - core_of * SLICE
    tile_all = dloc_all // 128
    half_all = (src >= HALF).astype(np.int64)
    counts_grid = np.zeros((NCORES, TILES, 2), np.int64)
    for k in range(NCORES):
        m = core_of == k
        np.add.at(counts_grid[k], (tile_all[m], half_all[m]), 1)
    blocks_grid = np.maximum((np.ceil(counts_grid.max(axis=0) / 128)).astype(np.int64), 1)
    NB = int(blocks_grid.sum())

    order = _layout_order()
    # linear sort key per (t, h) following the layout order
    lin_of = np.zeros((TILES, 2), np.int64)
    for i, (t, h) in enumerate(order):
        lin_of[t, h] = i
    # block offset of each (t, h) in the layout
    boff_of = np.zeros((TILES, 2), np.int64)
    off = 0
    for t, h in order:
        boff_of[t, h] = off
        off += int(blocks_grid[t, h])

    # per-core max in-degree (over virtual 5120 rows) for the deg_in pad
    P_in = 8
    percore_masks = []
    for k in range(NCORES):
        m = core_of == k
        percore_masks.append(m)
        cnt_in = np.bincount(dloc_all[m], minlength=5120)
        P_in = max(P_in, ((int(cnt_in.max()) + 7) // 8) * 8)

    per_core = []
    for k in range(NCORES):
        m = percore_masks[k]
        s_k, w_k = src[m], w[m]
        t_k, h_k, dl_k = tile_all[m], half_all[m], dloc_all[m]

        cnt_in = np.bincount(dl_k, minlength=5120)
        deg_in_pad = np.zeros((5120, P_in), np.float32)
        order_d = np.argsort(dl_k, kind="stable")
        sdl, swk = dl_k[order_d], w_k[order_d]
        st = np.zeros(5120, np.int64)
        st[1:] = np.cumsum(cnt_in)[:-1]
        deg_in_pad[sdl, np.arange(len(sdl)) - st[sdl]] = swk
        deg_in_cols = deg_in_pad.reshape(TILES, 128, P_in).transpose(1, 0, 2).reshape(
            128, TILES * P_in).astype(BF16)

        gidx = np.zeros(NB * 128, np.int64)
        wcol = np.zeros(NB * 128, np.float32)
        dcol = np.zeros(NB * 128, np.float32)
        key = lin_of[t_k, h_k]
        eorder = np.argsort(key, kind="stable")
        s_o, w_o, d_o = s_k[eorder], w_k[eorder], dl_k[eorder]
        key_o = key[eorder]
        starts_g = np.searchsorted(key_o, np.arange(TILES * 2))
        ends_g = np.searchsorted(key_o, np.arange(TILES * 2) + 1)
        for t, h in order:
            off = int(boff_of[t, h]) * 128
            a, b = starts_g[lin_of[t, h]], ends_g[lin_of[t, h]]
            n = b - a
            gidx[off:off + n] = s_o[a:b] - h * HALF
            wcol[off:off + n] = w_o[a:b]
            dcol[off:off + n] = d_o[a:b] - t * 128
        per_core.append({
            "gidx": _wrap_idx16(gidx),
            "wcol": wcol.reshape(NB, 128).T.copy(),
            "dcol": dcol.reshape(NB, 128).T.copy(),
            "deg_in": deg_in_cols,
        })
    return per_core, blocks_grid, deg_out_cols, P_out, P_in


def _build_kernel(shapes):
    import concourse.bass as bass  # noqa: F401
    import concourse.tile as tile
    from concourse import bacc, mybir

    f32 = mybir.dt.float32
    bf16 = mybir.dt.bfloat16
    ACT = mybir.ActivationFunctionType
    nc = bacc.Bacc("TRN2", target_bir_lowering=False, debug=False,
                   enable_asserts=False, num_devices=NCORES)

    tabs = {
        "com": nc.dram_tensor("com_emb", [N_COM, D], f32, kind="ExternalInput"),
        "pos": nc.dram_tensor("pos_emb", [N_POS, D], f32, kind="ExternalInput"),
    }
    ins = {}
    for rname, s_t, d_t in RELS:
        sh = shapes[rname]
        NB = int(sh["blocks_grid"].sum())
        ins[rname] = {
            "gidx": nc.dram_tensor(f"{rname}_gidx", [128, NB * 8], mybir.dt.int16,
                                   kind="ExternalInput"),
            "wcol": nc.dram_tensor(f"{rname}_wcol", [128, NB], f32, kind="ExternalInput"),
            "dcol": nc.dram_tensor(f"{rname}_dcol", [128, NB], f32, kind="ExternalInput"),
            "dout": nc.dram_tensor(f"{rname}_degout", [128, NT_TAB * sh["P_out"]], bf16,
                                   kind="ExternalInput"),
            "din": nc.dram_tensor(f"{rname}_degin", [128, TILES * sh["P_in"]], bf16,
                                  kind="ExternalInput"),
            "W": nc.dram_tensor(f"W_{rname}", [D, D], f32, kind="ExternalInput"),
            "b": nc.dram_tensor(f"b_{rname}", [1, D], f32, kind="ExternalInput"),
        }
    # paired scaled tables: row j = [r1*h_j (128 bf16) | r2*h_j (128 bf16)]
    # so chunk stores use 512B descriptors (full DMA rate) while each
    # relation's gather reads its 256B half via elem_step. Each table is
    # split into the two src-half tensors so h0 gathers only depend on the
    # first half of the build pass.
    spair = {nt: (nc.dram_tensor(f"spair_{nt}_h0", [HALF, 2 * D], bf16),
                  nc.dram_tensor(f"spair_{nt}_h1", [H1R, 2 * D], bf16))
             for nt in ("com", "pos")}
    rel_half = {}
    for nt, rels2 in AUGS.items():
        for ri, rel in enumerate(rels2):
            rel_half[rel] = (nt, ri)
    out = nc.dram_tensor("out", [2, SLICE, D], f32, kind="ExternalOutput")

    order = _layout_order()
    # group metadata per relation: (t,h) -> block offset; (tg,h) -> (off, gnb)
    meta = {}
    for rname, s_t, d_t in RELS:
        bg = shapes[rname]["blocks_grid"]
        boff_of = {}
        off = 0
        for t, h in order:
            boff_of[(t, h)] = off
            off += int(bg[t, h])
        grp = {}
        for tg in range(NGRP):
            ts = range(tg * TGRP, (tg + 1) * TGRP)
            for h in range(2):
                goff = boff_of[(ts[0], h)]
                gnb = sum(int(bg[t, h]) for t in ts)
                grp[(tg, h)] = (goff, gnb)
        meta[rname] = (boff_of, grp)
    GB_MAX = max(gnb for rname in meta for (goff, gnb) in meta[rname][1].values())
    DOCHUNK = 64

    with tile.TileContext(nc) as tc:
        with ExitStack() as ctx:
            const_p = ctx.enter_context(tc.tile_pool(name="const", bufs=1))
            deg_p = ctx.enter_context(tc.tile_pool(name="deg", bufs=2))
            stage_p = ctx.enter_context(tc.tile_pool(name="stage", bufs=2))
            idxp = ctx.enter_context(tc.tile_pool(name="idx", bufs=1))
            gp = ctx.enter_context(tc.tile_pool(name="g", bufs=3))
            ohp = ctx.enter_context(tc.tile_pool(name="oh", bufs=4))
            psp = ctx.enter_context(tc.tile_pool(name="ps", bufs=2, space="PSUM"))
            ps2 = ctx.enter_context(tc.tile_pool(name="ps2", bufs=2, space="PSUM"))
            ep = ctx.enter_context(tc.tile_pool(name="ep", bufs=3))
            keep = ctx.enter_context(tc.tile_pool(name="keep", bufs=1))

            # constants: iota row 0..127 (every partition), identity matrix
            iota_i = const_p.tile([128, 128], mybir.dt.int32)
            nc.gpsimd.iota(iota_i[:], pattern=[[1, 128]], base=0, channel_multiplier=0)
            iota_bf = const_p.tile([128, 128], bf16)
            nc.vector.tensor_copy(iota_bf[:], iota_i[:])
            pidx_i = const_p.tile([128, 1], mybir.dt.int32)
            nc.gpsimd.iota(pidx_i[:], pattern=[[1, 1]], base=0, channel_multiplier=1)
            pidx_f = const_p.tile([128, 1], f32)
            nc.vector.tensor_copy(pidx_f[:], pidx_i[:])
            ident_bf = const_p.tile([128, 128], bf16)
            nc.vector.tensor_scalar(ident_bf[:], iota_bf[:], pidx_f[:], None,
                                    op0=mybir.AluOpType.is_equal)

            # --- phase 1 helpers: degrees -> rsqrt ---
            r_in = {}
            r_out = {}

            def deg_out_phase(rname):
                sh = shapes[rname]
                P_out = sh["P_out"]
                inr = ins[rname]
                ro_t = keep.tile([128, NT_TAB], f32, tag=f"rq_{rname}",
                                 name=f"rq_{rname}")
                rsum = deg_p.tile([128, NT_TAB], bf16, tag="rsum")
                dov = inr["dout"].ap().rearrange("p (t q) -> p t q", q=P_out)
                for c0 in range(0, NT_TAB, DOCHUNK):
                    cn = min(DOCHUNK, NT_TAB - c0)
                    do_t = deg_p.tile([128, DOCHUNK * P_out], bf16, tag="dout")
                    dv = do_t[:].rearrange("p (t q) -> p t q", q=P_out)
                    nc.sync.dma_start(dv[:, 0:cn, :], dov[:, c0:c0 + cn, :])
                    # deg values are O(25) sums of O(25) uniform weights;
                    # bf16 accumulation error (~0.5%) is fine for the rsqrt
                    # norm and gets the 2x 16-bit DVE rate.
                    with nc.allow_low_precision(reason="degree norm tolerates bf16"):
                        nc.vector.reduce_sum(rsum[:, c0:c0 + cn], dv[:, 0:cn, :],
                                             axis=mybir.AxisListType.X)
                nc.vector.tensor_scalar_max(ro_t[:], rsum[:], EPS)
                nc.scalar.activation(ro_t[:], ro_t[:], ACT.Sqrt)
                nc.vector.reciprocal(ro_t[:], ro_t[:])
                r_out[rname] = ro_t

            def deg_in_phase(rname):
                sh = shapes[rname]
                P_in = sh["P_in"]
                inr = ins[rname]
                di_t = deg_p.tile([128, TILES * P_in], bf16, tag="din")
                nc.sync.dma_start(di_t[:], inr["din"].ap())
                ri_t = keep.tile([128, TILES], f32, tag=f"rin_{rname}",
                                 name=f"rin_{rname}")
                nc.vector.reduce_sum(ri_t[:],
                                     di_t[:].rearrange("p (t q) -> p t q", q=P_in),
                                     axis=mybir.AxisListType.X)
                nc.vector.tensor_scalar_max(ri_t[:], ri_t[:], EPS)
                nc.scalar.activation(ri_t[:], ri_t[:], ACT.Sqrt)
                nc.vector.reciprocal(ri_t[:], ri_t[:])
                r_in[rname] = ri_t

            # --- phase 2 helper: one shared fp32 read pass per source table
            # builds both relations' r_out-scaled bf16 halves of the paired
            # split tables. Returns a list of per-chunk emitters so callers
            # can interleave the build with edge-phase emission. The two
            # relations' scale-casts split across ACT and DVE. ---
            def build_chunks(nt):
                rels2 = AUGS[nt]

                def mk_chunk(hh, jloc, g0, jn):
                    def emit():
                        rawv = tabs[nt].ap()[g0 * 128:(g0 + jn) * 128, :].rearrange(
                            "(j p) d -> p j d", p=128)
                        spv = spair[nt][hh].ap()[jloc * 128:(jloc + jn) * 128, :
                                                 ].rearrange("(j p) d -> p j d", p=128)
                        f32st = stage_p.tile([128, CH * D], f32, tag="f32st")
                        fv = f32st[:].rearrange("p (j d) -> p j d", d=D)
                        nc.sync.dma_start(fv[:, 0:jn, :], rawv[:])
                        bfst = stage_p.tile([128, CH * 2 * D], bf16, tag="bfst")
                        bv = bfst[:].rearrange("p (j d) -> p j d", d=2 * D)
                        for j in range(jn):
                            nc.scalar.activation(
                                bv[:, j, 0:D], fv[:, j, :], ACT.Copy,
                                scale=r_out[rels2[0]][:, g0 + j:g0 + j + 1])
                            nc.vector.tensor_scalar_mul(
                                bv[:, j, D:2 * D], fv[:, j, :],
                                r_out[rels2[1]][:, g0 + j:g0 + j + 1])
                        nc.sync.dma_start(spv[:], bv[:, 0:jn, :])
                    return emit

                chunks = []
                for hh, (t0, ntile) in enumerate(((0, H0T), (H0T, 312 - H0T))):
                    for j0 in range(0, ntile, CH):
                        jn = min(CH, ntile - j0)
                        chunks.append(mk_chunk(hh, j0, t0 + j0, jn))

                def emit_partial():
                    # last tile (rows 39936..40000, 64 rows -> h1 local rows)
                    f32p = stage_p.tile([128, D], f32, tag="f32p")
                    nc.sync.dma_start(f32p[0:64, :], tabs[nt].ap()[312 * 128:N_COM, :])
                    bfp = stage_p.tile([128, 2 * D], bf16, tag="bfp")
                    nc.scalar.activation(bfp[0:64, 0:D], f32p[0:64, :],
                                         ACT.Copy, scale=r_out[rels2[0]][0:64, 312:313])
                    nc.vector.tensor_scalar_mul(bfp[0:64, D:2 * D], f32p[0:64, :],
                                                r_out[rels2[1]][0:64, 312:313])
                    lo = 312 * 128 - HALF
                    nc.sync.dma_start(spair[nt][1].ap()[lo:lo + 64, :], bfp[0:64, :])
                chunks.append(emit_partial)
                return chunks

            # --- phase 3 helper: per relation edge gather + scatter matmul ---
            acc_out = {
                "com": keep.tile([128, TILES * D], f32, tag="acc_com", name="acc_com"),
                "pos": keep.tile([128, TILES * D], f32, tag="acc_pos", name="acc_pos"),
            }
            first_rel = {"com": True, "pos": True}

            def edge_phase(rname, s_t, d_t, interleave=None):
                sh = shapes[rname]
                bg = sh["blocks_grid"]
                NB = int(bg.sum())
                inr = ins[rname]
                boff_of, grp = meta[rname]

                gidx_t = idxp.tile([128, NB * 8], mybir.dt.int16, tag="gidx")
                nc.sync.dma_start(gidx_t[:], inr["gidx"].ap())
                wcol_t = idxp.tile([128, NB], f32, tag="wcol")
                nc.sync.dma_start(wcol_t[:], inr["wcol"].ap())
                dcol_t = idxp.tile([128, NB], f32, tag="dcol")
                nc.sync.dma_start(dcol_t[:], inr["dcol"].ap())

                W_f = const_p.tile([128, D], f32, tag=f"Wf_{rname}")
                nc.sync.dma_start(W_f[:], inr["W"].ap())
                W_bf = const_p.tile([128, D], bf16, tag=f"Wb_{rname}")
                nc.vector.tensor_copy(W_bf[:], W_f[:])
                b_row = const_p.tile([1, D], f32, tag=f"b_{rname}")
                nc.sync.dma_start(b_row[:], inr["b"].ap())
                b_rep = const_p.tile([128, D], f32, tag=f"brep_{rname}")
                nc.gpsimd.partition_broadcast(b_rep[:], b_row[:])

                nt_h, ri_h = rel_half[rname]
                c0 = ri_h * D
                half_views = [spair[nt_h][0].ap()[0:HALF, c0:c0 + D],
                              spair[nt_h][1].ap()[0:H1R, c0:c0 + D]]
                ri_t = r_in[rname]
                acc = acc_out[d_t]

                for tg in range(NGRP):
                    if interleave:
                        for _ in range(3):
                            if interleave:
                                interleave.pop(0)()
                    ts = list(range(tg * TGRP, (tg + 1) * TGRP))
                    gviews = {}
                    for h in range(2):
                        goff, gnb = grp[(tg, h)]
                        g = gp.tile([128, GB_MAX * D], bf16, tag=f"g{h}")
                        gv = g[:].rearrange("p (b d) -> p b d", d=D)
                        ni = gnb * 128
                        nc.gpsimd.dma_gather(
                            gv[:, 0:gnb, :], half_views[h],
                            gidx_t[:, goff * 8:(goff + gnb) * 8],
                            num_idxs=ni, num_idxs_reg=ni, elem_size=D,
                            elem_step=2 * D, single_packet=False)
                        gviews[h] = gv
                    for t in ts:
                        ps = psp.tile([128, D], f32, tag="acc")
                        nblocks = [int(bg[t, h]) for h in range(2)]
                        first = True
                        for h in range(2):
                            goff, gnb = grp[(tg, h)]
                            lo = boff_of[(t, h)] - goff
                            gv = gviews[h]
                            for bi in range(nblocks[h]):
                                col = boff_of[(t, h)] + bi
                                slot = lo + bi
                                oh = ohp.tile([128, 128], bf16, tag="oh")
                                nc.vector.tensor_scalar(
                                    oh[:], iota_bf[:],
                                    dcol_t[:, col:col + 1], wcol_t[:, col:col + 1],
                                    op0=mybir.AluOpType.is_equal,
                                    op1=mybir.AluOpType.mult)
                                nc.tensor.matmul(
                                    ps[:], oh[:], gv[:, slot:slot + 1, 0:D],
                                    start=first,
                                    stop=(h == 1 and bi == nblocks[1] - 1))
                                first = False
                        # epilogue: Y = relu((rin*G) @ W + b); acc += 0.5*Y
                        gn = ep.tile([128, D], bf16, tag="gn")
                        nc.scalar.activation(gn[:], ps[:], ACT.Copy,
                                             scale=ri_t[:, t:t + 1])
                        gT_ps = ps2.tile([128, D], bf16, tag="gT")
                        nc.tensor.transpose(gT_ps[:], gn[:], ident_bf[:])
                        gT = ep.tile([128, D], bf16, tag="gTs")
                        nc.scalar.activation(gT[:], gT_ps[:], ACT.Copy)
                        y_ps = ps2.tile([128, D], f32, tag="y")
                        nc.tensor.matmul(y_ps[:], gT[:], W_bf[:], start=True, stop=True)
                        tmp = ep.tile([128, D], f32, tag="tmp")
                        nc.vector.tensor_add(tmp[:], y_ps[:], b_rep[:])
                        if first_rel[d_t]:
                            nc.scalar.activation(acc[:, t * D:(t + 1) * D], tmp[:],
                                                 ACT.Relu, scale=0.5)
                        else:
                            tmp2 = ep.tile([128, D], f32, tag="tmp2")
                            nc.scalar.activation(tmp2[:], tmp[:], ACT.Relu, scale=0.5)
                            nc.vector.tensor_add(
                                acc[:, t * D:(t + 1) * D],
                                acc[:, t * D:(t + 1) * D], tmp2[:])
                first_rel[d_t] = False

            # Emission order: only demand+cflow deg_out gates the com build;
            # the pos-table build chunks interleave into demand's edge loop
            # so its scale pass hides under demand's gather DMA.
            deg_out_phase("demand")
            deg_out_phase("cflow")
            for emit in build_chunks("com"):
                emit()
            deg_out_phase("supply")
            deg_out_phase("pflow")
            for r, _, _ in RELS:
                deg_in_phase(r)
            def store_out(i, ntype):
                acc = acc_out[ntype]
                accv = acc[:, 0:39 * D].rearrange("p (t d) -> p t d", d=D)
                outv = out.ap()[i, 0:39 * 128, :].rearrange("(t p) d -> p t d", p=128)
                nc.sync.dma_start(outv[:], accv[:])
                nc.sync.dma_start(out.ap()[i, 39 * 128:SLICE, :],
                                  acc[0:8, 39 * D:40 * D])

            pos_chunks = build_chunks("pos")
            edge_phase(*RELS[0], interleave=pos_chunks)   # demand (spair_com)
            for emit in pos_chunks:   # any leftovers
                emit()
            pos_chunks.clear()
            edge_phase(*RELS[1])   # cflow (reads spair_com)
            edge_phase(*RELS[2])   # supply (reads spair_pos)
            store_out(0, "com")    # acc_com final after supply
            edge_phase(*RELS[3])   # pflow (reads spair_pos)
            store_out(1, "pos")
    nc.compile()
    return nc


LAST_RESULT = None


def kernel(**inputs):
    global LAST_RESULT
    from concourse.bass_utils import run_bass_kernel_spmd

    com_emb = np.asarray(inputs["com_emb"], np.float32)
    pos_emb = np.asarray(inputs["pos_emb"], np.float32)

    shapes, percore_rel = {}, {}
    for rname, s_t, d_t in RELS:
        per_core, blocks_grid, deg_out_cols, P_out, P_in = _prep_relation(
            inputs[f"{rname}_src"], inputs[f"{rname}_dst"], inputs[f"{rname}_w"])
        shapes[rname] = {"blocks_grid": blocks_grid, "P_out": P_out, "P_in": P_in}
        percore_rel[rname] = (per_core, deg_out_cols)

    nc = _build_kernel(shapes)

    in_maps = []
    for k in range(NCORES):
        m = {"com_emb": com_emb, "pos_emb": pos_emb}
        for rname, s_t, d_t in RELS:
            per_core, deg_out_cols = percore_rel[rname]
            pc = per_core[k]
            m[f"{rname}_gidx"] = pc["gidx"]
            m[f"{rname}_wcol"] = pc["wcol"]
            m[f"{rname}_dcol"] = pc["dcol"]
            m[f"{rname}_degout"] = deg_out_cols
            m[f"{rname}_degin"] = pc["deg_in"]
            m[f"W_{rname}"] = np.asarray(inputs[f"W_{rname}"], np.float32)
            m[f"b_{rname}"] = np.asarray(inputs[f"b_{rname}"], np.float32).reshape(1, D)
        in_maps.append(m)

    res = run_bass_kernel_spmd(nc, in_maps, core_ids=list(range(NCORES)))
    LAST_RESULT = res
    out = np.empty((2, N_COM, D), np.float32)
    for k in range(NCORES):
        o = res.results[k]["out"]
        out[0, k * SLICE:(k + 1) * SLICE] = o[0]
        out[1, k * SLICE:(k + 1) * SLICE] = o[1]
    return out


# revision 42
# speedup vs baseline: 13.8012x; 2.1087x over previous
"""ComPosHGNN Trainium2 kernel: 4-relation heterogeneous GraphConv.

Sharding: each relation's edges are bucketed by DESTINATION range (5000
nodes/core, 8 cores) -- every core computes its own slice of both output
node types, so no collectives are needed.  Host work is layout only
(bucket/sort/pad); all arithmetic (degrees, normalization, projection,
aggregation, relu) runs on device.

Device pipeline (v2 -- bf16 augmented-table design):
  1. deg_out/deg_in from host-padded bf16 weight layouts via reduce_sum,
     then rsqrt (clamped at eps; padding slots have w=0 so exact).
  2. Per SOURCE TABLE (com/pos): ONE shared fp32 read pass builds BOTH
     relations' r_out-scaled bf16 tables (256B rows) in HBM scratch --
     big chunked DMAs, ACT-engine scale+cast (per-partition scale).
  3. Per relation, per 2-dst-tile group and src half: one dma_gather of
     the (host-padded) edge blocks' 256B scaled rows.
  4. Per 128-dst tile: build w-scaled one-hot matrices (bf16) via
     iota-compare; TensorE matmuls scatter-accumulate
     G_t = sum_e w_e*h~[src_e] into fp32 PSUM.
  5. Epilogue: scale rows by rsqrt(deg_in) (ACT), transpose (PE),
     project by W (PE, bf16), add bias, relu (ACT), average the two
     relations per output ntype.
"""
import numpy as np
import ml_dtypes
from contextlib import ExitStack

BF16 = ml_dtypes.bfloat16

N_COM = 40000
N_POS = 40000
D = 128
NCORES = 8
SLICE = N_COM // NCORES          # 5000 dst nodes per core
TILES = 40                       # 39 full 128-row tiles + 1 partial (8 rows)
NT_TAB = 313                     # table tiles (40064 = 313*128 padded rows)
NPAD = NT_TAB * 128
HALF = 19968                     # src-half split (128-aligned) for int16 gather
H0T = HALF // 128                # 156 table tiles in half 0
H1R = NPAD - HALF                # 20096 rows in half 1 (157 tiles, last partial)
EPS = 1e-20
TGRP = 2                         # dst tiles per gather group
NGRP = TILES // TGRP
CH = 8                           # table tiles per scale-pass chunk (312 = 39*8)

# relation -> (src table, dst ntype)
RELS = [
    ("demand", "com", "pos"),
    ("cflow", "com", "com"),
    ("supply", "pos", "com"),
    ("pflow", "pos", "pos"),
]
# src table -> the two relations whose scaled copies share its read pass
AUGS = {"com": ("demand", "cflow"), "pos": ("supply", "pflow")}


def _layout_order():
    """Block-group layout: [tile-group][src-half][tile-within-group]."""
    order = []
    for tg in range(NGRP):
        for h in range(2):
            for t in range(tg * TGRP, (tg + 1) * TGRP):
                order.append((t, h))
    return order


def _wrap_idx16(idx):
    """dma_gather index layout: idx i at [i%16, i//16], tiled x8 (Q7 cores)."""
    assert len(idx) % 16 == 0
    m = idx.astype(np.int16).reshape(-1, 16).T
    return np.tile(m, (8, 1))


def _prep_relation(src, dst, w):
    """Host-side layout for one relation (all cores): gather indices,
    per-block w/dst_local columns, and padded degree arrays."""
    src = np.asarray(src, np.int64)
    dst = np.asarray(dst, np.int64)
    w = np.asarray(w, np.float32)

    # padded-by-src weight array for deg_out (global, shared by all cores)
    counts_s = np.bincount(src, minlength=NPAD)
    P_out = max(8, ((int(counts_s.max()) + 7) // 8) * 8)
    deg_out_pad = np.zeros((NPAD, P_out), np.float32)
    order_s = np.argsort(src, kind="stable")
    ssrc, sw = src[order_s], w[order_s]
    starts = np.zeros(NPAD, np.int64)
    starts[1:] = np.cumsum(counts_s)[:-1]
    deg_out_pad[ssrc, np.arange(len(ssrc)) - starts[ssrc]] = sw
    deg_out_cols = deg_out_pad.reshape(NT_TAB, 128, P_out).transpose(1, 0, 2).reshape(
        128, NT_TAB * P_out).astype(BF16)

    core_of = dst // SLICE
    dloc_all = dst - core_of * SLICE
    tile_all = dloc_all // 128
    half_all = (src >= HALF).astype(np.int64)
    counts_grid = np.zeros((NCORES, TILES, 2), np.int64)
    for k in range(NCORES):
        m = core_of == k
        np.add.at(counts_grid[k], (tile_all[m], half_all[m]), 1)
    blocks_grid = np.maximum((np.ceil(counts_grid.max(axis=0) / 128)).astype(np.int64), 1)
    NB = int(blocks_grid.sum())

    order = _layout_order()
    # linear sort key per (t, h) following the layout order
    lin_of = np.zeros((TILES, 2), np.int64)
    for i, (t, h) in enumerate(order):
        lin_of[t, h] = i
    # block offset of each (t, h) in the layout
    boff_of = np.zeros((TILES, 2), np.int64)
    off = 0
    for t, h in order:
        boff_of[t, h] = off
        off += int(blocks_grid[t, h])

    # per-core max in-degree (over virtual 5120 rows) for the deg_in pad
    P_in = 8
    percore_masks = []
    for k in range(NCORES):
        m = core_of == k
        percore_masks.append(m)
        cnt_in = np.bincount(dloc_all[m], minlength=5120)
        P_in = max(P_in, ((int(cnt_in.max()) + 7) // 8) * 8)

    per_core = []
    for k in range(NCORES):
        m = percore_masks[k]
        s_k, w_k = src[m], w[m]
        t_k, h_k, dl_k = tile_all[m], half_all[m], dloc_all[m]

        cnt_in = np.bincount(dl_k, minlength=5120)
        deg_in_pad = np.zeros((5120, P_in), np.float32)
        order_d = np.argsort(dl_k, kind="stable")
        sdl, swk = dl_k[order_d], w_k[order_d]
        st = np.zeros(5120, np.int64)
        st[1:] = np.cumsum(cnt_in)[:-1]
        deg_in_pad[sdl, np.arange(len(sdl)) - st[sdl]] = swk
        deg_in_cols = deg_in_pad.reshape(TILES, 128, P_in).transpose(1, 0, 2).reshape(
            128, TILES * P_in).astype(BF16)

        gidx = np.zeros(NB * 128, np.int64)
        wcol = np.zeros(NB * 128, np.float32)
        dcol = np.zeros(NB * 128, np.float32)
        key = lin_of[t_k, h_k]
        eorder = np.argsort(key, kind="stable")
        s_o, w_o, d_o = s_k[eorder], w_k[eorder], dl_k[eorder]
        key_o = key[eorder]
        starts_g = np.searchsorted(key_o, np.arange(TILES * 2))
        ends_g = np.searchsorted(key_o, np.arange(TILES * 2) + 1)
        for t, h in order:
            off = int(boff_of[t, h]) * 128
            a, b = starts_g[lin_of[t, h]], ends_g[lin_of[t, h]]
            n = b - a
            gidx[off:off + n] = s_o[a:b] - h * HALF
            wcol[off:off + n] = w_o[a:b]
            dcol[off:off + n] = d_o[a:b] - t * 128
        per_core.append({
            "gidx": _wrap_idx16(gidx),
            "wcol": wcol.reshape(NB, 128).T.copy(),
            "dcol": dcol.reshape(NB, 128).T.copy(),
            "deg_in": deg_in_cols,
        })
    return per_core, blocks_grid, deg_out_cols, P_out, P_in


def _build_kernel(shapes):
    import concourse.bass as bass  # noqa: F401
    import concourse.tile as tile
    from concourse import bacc, mybir

    f32 = mybir.dt.float32
    bf16 = mybir.dt.bfloat16
    ACT = mybir.ActivationFunctionType
    nc = bacc.Bacc("TRN2", target_bir_lowering=False, debug=False,
                   enable_asserts=False, num_devices=NCORES)

    tabs = {
        "com": nc.dram_tensor("com_emb", [N_COM, D], f32, kind="ExternalInput"),
        "pos": nc.dram_tensor("pos_emb", [N_POS, D], f32, kind="ExternalInput"),
    }
    ins = {}
    for rname, s_t, d_t in RELS:
        sh = shapes[rname]
        NB = int(sh["blocks_grid"].sum())
        ins[rname] = {
            "gidx": nc.dram_tensor(f"{rname}_gidx", [128, NB * 8], mybir.dt.int16,
                                   kind="ExternalInput"),
            "wcol": nc.dram_tensor(f"{rname}_wcol", [128, NB], f32, kind="ExternalInput"),
            "dcol": nc.dram_tensor(f"{rname}_dcol", [128, NB], f32, kind="ExternalInput"),
            "dout": nc.dram_tensor(f"{rname}_degout", [128, NT_TAB * sh["P_out"]], bf16,
                                   kind="ExternalInput"),
            "din": nc.dram_tensor(f"{rname}_degin", [128, TILES * sh["P_in"]], bf16,
                                  kind="ExternalInput"),
            "W": nc.dram_tensor(f"W_{rname}", [D, D], f32, kind="ExternalInput"),
            "b": nc.dram_tensor(f"b_{rname}", [1, D], f32, kind="ExternalInput"),
        }
    # paired scaled tables: row j = [r1*h_j (128 bf16) | r2*h_j (128 bf16)]
    # so chunk stores use 512B descriptors (full DMA rate) while each
    # relation's gather reads its 256B half via elem_step. Each table is
    # split into the two src-half tensors so h0 gathers only depend on the
    # first half of the build pass.
    spair = {nt: (nc.dram_tensor(f"spair_{nt}_h0", [HALF, 2 * D], bf16),
                  nc.dram_tensor(f"spair_{nt}_h1", [H1R, 2 * D], bf16))
             for nt in ("com", "pos")}
    rel_half = {}
    for nt, rels2 in AUGS.items():
        for ri, rel in enumerate(rels2):
            rel_half[rel] = (nt, ri)
    out = nc.dram_tensor("out", [2, SLICE, D], f32, kind="ExternalOutput")

    order = _layout_order()
    # group metadata per relation: (t,h) -> block offset; (tg,h) -> (off, gnb)
    meta = {}
    for rname, s_t, d_t in RELS:
        bg = shapes[rname]["blocks_grid"]
        boff_of = {}
        off = 0
        for t, h in order:
            boff_of[(t, h)] = off
            off += int(bg[t, h])
        grp = {}
        for tg in range(NGRP):
            ts = range(tg * TGRP, (tg + 1) * TGRP)
            for h in range(2):
                goff = boff_of[(ts[0], h)]
                gnb = sum(int(bg[t, h]) for t in ts)
                grp[(tg, h)] = (goff, gnb)
        meta[rname] = (boff_of, grp)
    GB_MAX = max(gnb for rname in meta for (goff, gnb) in meta[rname][1].values())
    DOCHUNK = 64

    with tile.TileContext(nc) as tc:
        with ExitStack() as ctx:
            const_p = ctx.enter_context(tc.tile_pool(name="const", bufs=1))
            deg_p = ctx.enter_context(tc.tile_pool(name="deg", bufs=2))
            stage_p = ctx.enter_context(tc.tile_pool(name="stage", bufs=3))
            idxp = ctx.enter_context(tc.tile_pool(name="idx", bufs=1))
            gp = ctx.enter_context(tc.tile_pool(name="g", bufs=3))
            ohp = ctx.enter_context(tc.tile_pool(name="oh", bufs=4))
            psp = ctx.enter_context(tc.tile_pool(name="ps", bufs=3, space="PSUM"))
            ps2 = ctx.enter_context(tc.tile_pool(name="ps2", bufs=2, space="PSUM"))
            ep = ctx.enter_context(tc.tile_pool(name="ep", bufs=3))
            keep = ctx.enter_context(tc.tile_pool(name="keep", bufs=1))

            # constants: iota row 0..127 (every partition), identity matrix
            iota_i = const_p.tile([128, 128], mybir.dt.int32)
            nc.gpsimd.iota(iota_i[:], pattern=[[1, 128]], base=0, channel_multiplier=0)
            iota_bf = const_p.tile([128, 128], bf16)
            nc.vector.tensor_copy(iota_bf[:], iota_i[:])
            pidx_i = const_p.tile([128, 1], mybir.dt.int32)
            nc.gpsimd.iota(pidx_i[:], pattern=[[1, 1]], base=0, channel_multiplier=1)
            pidx_f = const_p.tile([128, 1], f32)
            nc.vector.tensor_copy(pidx_f[:], pidx_i[:])
            ident_bf = const_p.tile([128, 128], bf16)
            nc.vector.tensor_scalar(ident_bf[:], iota_bf[:], pidx_f[:], None,
                                    op0=mybir.AluOpType.is_equal)

            # --- phase 1 helpers: degrees -> rsqrt ---
            r_in = {}
            r_out = {}

            def deg_out_phase(rname):
                sh = shapes[rname]
                P_out = sh["P_out"]
                inr = ins[rname]
                ro_t = keep.tile([128, NT_TAB], f32, tag=f"rq_{rname}",
                                 name=f"rq_{rname}")
                rsum = deg_p.tile([128, NT_TAB], bf16, tag="rsum")
                dov = inr["dout"].ap().rearrange("p (t q) -> p t q", q=P_out)
                for c0 in range(0, NT_TAB, DOCHUNK):
                    cn = min(DOCHUNK, NT_TAB - c0)
                    do_t = deg_p.tile([128, DOCHUNK * P_out], bf16, tag="dout")
                    dv = do_t[:].rearrange("p (t q) -> p t q", q=P_out)
                    nc.sync.dma_start(dv[:, 0:cn, :], dov[:, c0:c0 + cn, :])
                    # deg values are O(25) sums of O(25) uniform weights;
                    # bf16 accumulation error (~0.5%) is fine for the rsqrt
                    # norm and gets the 2x 16-bit DVE rate.
                    with nc.allow_low_precision(reason="degree norm tolerates bf16"):
                        nc.vector.reduce_sum(rsum[:, c0:c0 + cn], dv[:, 0:cn, :],
                                             axis=mybir.AxisListType.X)
                nc.vector.tensor_scalar_max(ro_t[:], rsum[:], EPS)
                nc.scalar.activation(ro_t[:], ro_t[:], ACT.Sqrt)
                nc.vector.reciprocal(ro_t[:], ro_t[:])
                r_out[rname] = ro_t

            def deg_in_phase(rname):
                sh = shapes[rname]
                P_in = sh["P_in"]
                inr = ins[rname]
                di_t = deg_p.tile([128, TILES * P_in], bf16, tag="din")
                nc.sync.dma_start(di_t[:], inr["din"].ap())
                ri_t = keep.tile([128, TILES], f32, tag=f"rin_{rname}",
                                 name=f"rin_{rname}")
                nc.vector.reduce_sum(ri_t[:],
                                     di_t[:].rearrange("p (t q) -> p t q", q=P_in),
                                     axis=mybir.AxisListType.X)
                nc.vector.tensor_scalar_max(ri_t[:], ri_t[:], EPS)
                nc.scalar.activation(ri_t[:], ri_t[:], ACT.Sqrt)
                nc.vector.reciprocal(ri_t[:], ri_t[:])
                r_in[rname] = ri_t

            # --- phase 2 helper: one shared fp32 read pass per source table
            # builds both relations' r_out-scaled bf16 halves of the paired
            # split tables. Returns a list of per-chunk emitters so callers
            # can interleave the build with edge-phase emission. The two
            # relations' scale-casts split across ACT and DVE. ---
            def build_chunks(nt):
                rels2 = AUGS[nt]

                def mk_chunk(hh, jloc, g0, jn):
                    def emit():
                        rawv = tabs[nt].ap()[g0 * 128:(g0 + jn) * 128, :].rearrange(
                            "(j p) d -> p j d", p=128)
                        spv = spair[nt][hh].ap()[jloc * 128:(jloc + jn) * 128, :
                                                 ].rearrange("(j p) d -> p j d", p=128)
                        f32st = stage_p.tile([128, CH * D], f32, tag="f32st")
                        fv = f32st[:].rearrange("p (j d) -> p j d", d=D)
                        nc.sync.dma_start(fv[:, 0:jn, :], rawv[:])
                        bfst = stage_p.tile([128, CH * 2 * D], bf16, tag="bfst")
                        bv = bfst[:].rearrange("p (j d) -> p j d", d=2 * D)
                        for j in range(jn):
                            nc.scalar.activation(
                                bv[:, j, 0:D], fv[:, j, :], ACT.Copy,
                                scale=r_out[rels2[0]][:, g0 + j:g0 + j + 1])
                            nc.vector.tensor_scalar_mul(
                                bv[:, j, D:2 * D], fv[:, j, :],
                                r_out[rels2[1]][:, g0 + j:g0 + j + 1])
                        nc.sync.dma_start(spv[:], bv[:, 0:jn, :])
                    return emit

                chunks = []
                for hh, (t0, ntile) in enumerate(((0, H0T), (H0T, 312 - H0T))):
                    for j0 in range(0, ntile, CH):
                        jn = min(CH, ntile - j0)
                        chunks.append(mk_chunk(hh, j0, t0 + j0, jn))

                def emit_partial():
                    # last tile (rows 39936..40000, 64 rows -> h1 local rows)
                    f32p = stage_p.tile([128, D], f32, tag="f32p")
                    nc.sync.dma_start(f32p[0:64, :], tabs[nt].ap()[312 * 128:N_COM, :])
                    bfp = stage_p.tile([128, 2 * D], bf16, tag="bfp")
                    nc.scalar.activation(bfp[0:64, 0:D], f32p[0:64, :],
                                         ACT.Copy, scale=r_out[rels2[0]][0:64, 312:313])
                    nc.vector.tensor_scalar_mul(bfp[0:64, D:2 * D], f32p[0:64, :],
                                                r_out[rels2[1]][0:64, 312:313])
                    lo = 312 * 128 - HALF
                    nc.sync.dma_start(spair[nt][1].ap()[lo:lo + 64, :], bfp[0:64, :])
                chunks.append(emit_partial)
                return chunks

            # --- phase 3 helper: per relation edge gather + scatter matmul ---
            acc_out = {
                "com": keep.tile([128, TILES * D], f32, tag="acc_com", name="acc_com"),
                "pos": keep.tile([128, TILES * D], f32, tag="acc_pos", name="acc_pos"),
            }
            first_rel = {"com": True, "pos": True}

            def edge_phase(rname, s_t, d_t, interleave=None):
                sh = shapes[rname]
                bg = sh["blocks_grid"]
                NB = int(bg.sum())
                inr = ins[rname]
                boff_of, grp = meta[rname]

                gidx_t = idxp.tile([128, NB * 8], mybir.dt.int16, tag="gidx")
                nc.sync.dma_start(gidx_t[:], inr["gidx"].ap())
                wcol_t = idxp.tile([128, NB], f32, tag="wcol")
                nc.sync.dma_start(wcol_t[:], inr["wcol"].ap())
                dcol_t = idxp.tile([128, NB], f32, tag="dcol")
                nc.sync.dma_start(dcol_t[:], inr["dcol"].ap())

                W_f = const_p.tile([128, D], f32, tag=f"Wf_{rname}")
                nc.sync.dma_start(W_f[:], inr["W"].ap())
                W_bf = const_p.tile([128, D], bf16, tag=f"Wb_{rname}")
                nc.vector.tensor_copy(W_bf[:], W_f[:])
                b_row = const_p.tile([1, D], f32, tag=f"b_{rname}")
                nc.sync.dma_start(b_row[:], inr["b"].ap())
                b_rep = const_p.tile([128, D], f32, tag=f"brep_{rname}")
                nc.gpsimd.partition_broadcast(b_rep[:], b_row[:])

                nt_h, ri_h = rel_half[rname]
                c0 = ri_h * D
                half_views = [spair[nt_h][0].ap()[0:HALF, c0:c0 + D],
                              spair[nt_h][1].ap()[0:H1R, c0:c0 + D]]
                ri_t = r_in[rname]
                acc = acc_out[d_t]

                for tg in range(NGRP):
                    if interleave:
                        for _ in range(3):
                            if interleave:
                                interleave.pop(0)()
                    ts = list(range(tg * TGRP, (tg + 1) * TGRP))
                    gviews = {}
                    for h in range(2):
                        goff, gnb = grp[(tg, h)]
                        g = gp.tile([128, GB_MAX * D], bf16, tag=f"g{h}")
                        gv = g[:].rearrange("p (b d) -> p b d", d=D)
                        ni = gnb * 128
                        nc.gpsimd.dma_gather(
                            gv[:, 0:gnb, :], half_views[h],
                            gidx_t[:, goff * 8:(goff + gnb) * 8],
                            num_idxs=ni, num_idxs_reg=ni, elem_size=D,
                            elem_step=2 * D, single_packet=False)
                        gviews[h] = gv
                    for t in ts:
                        ps = psp.tile([128, D], f32, tag="acc")
                        nblocks = [int(bg[t, h]) for h in range(2)]
                        first = True
                        for h in range(2):
                            goff, gnb = grp[(tg, h)]
                            lo = boff_of[(t, h)] - goff
                            gv = gviews[h]
                            for bi in range(nblocks[h]):
                                col = boff_of[(t, h)] + bi
                                slot = lo + bi
                                oh = ohp.tile([128, 128], bf16, tag="oh")
                                nc.vector.tensor_scalar(
                                    oh[:], iota_bf[:],
                                    dcol_t[:, col:col + 1], wcol_t[:, col:col + 1],
                                    op0=mybir.AluOpType.is_equal,
                                    op1=mybir.AluOpType.mult)
                                nc.tensor.matmul(
                                    ps[:], oh[:], gv[:, slot:slot + 1, 0:D],
                                    start=first,
                                    stop=(h == 1 and bi == nblocks[1] - 1))
                                first = False
                        # epilogue: Y = relu((rin*G) @ W + b); acc += 0.5*Y
                        gn = ep.tile([128, D], bf16, tag="gn")
                        nc.scalar.activation(gn[:], ps[:], ACT.Copy,
                                             scale=ri_t[:, t:t + 1])
                        gT_ps = ps2.tile([128, D], bf16, tag="gT")
                        nc.tensor.transpose(gT_ps[:], gn[:], ident_bf[:])
                        gT = ep.tile([128, D], bf16, tag="gTs")
                        nc.scalar.activation(gT[:], gT_ps[:], ACT.Copy)
                        y_ps = ps2.tile([128, D], f32, tag="y")
                        nc.tensor.matmul(y_ps[:], gT[:], W_bf[:], start=True, stop=True)
                        tmp = ep.tile([128, D], f32, tag="tmp")
                        nc.vector.tensor_add(tmp[:], y_ps[:], b_rep[:])
                        if first_rel[d_t]:
                            nc.scalar.activation(acc[:, t * D:(t + 1) * D], tmp[:],
                                                 ACT.Relu, scale=0.5)
                        else:
                            tmp2 = ep.tile([128, D], f32, tag="tmp2")
                            nc.scalar.activation(tmp2[:], tmp[:], ACT.Relu, scale=0.5)
                            nc.vector.tensor_add(
                                acc[:, t * D:(t + 1) * D],
                                acc[:, t * D:(t + 1) * D], tmp2[:])
                first_rel[d_t] = False

            # Emission order: only demand+cflow deg_out gates the com build;
            # the pos-table build chunks interleave into demand's edge loop
            # so its scale pass hides under demand's gather DMA.
            deg_out_phase("demand")
            deg_out_phase("cflow")
            for emit in build_chunks("com"):
                emit()
            deg_out_phase("supply")
            deg_out_phase("pflow")
            for r, _, _ in RELS:
                deg_in_phase(r)
            def store_out(i, ntype):
                acc = acc_out[ntype]
                accv = acc[:, 0:39 * D].rearrange("p (t d) -> p t d", d=D)
                outv = out.ap()[i, 0:39 * 128, :].rearrange("(t p) d -> p t d", p=128)
                nc.sync.dma_start(outv[:], accv[:])
                nc.sync.dma_start(out.ap()[i, 39 * 128:SLICE, :],
                                  acc[0:8, 39 * D:40 * D])

            pos_chunks = build_chunks("pos")
            edge_phase(*RELS[0], interleave=pos_chunks)   # demand (spair_com)
            for emit in pos_chunks:   # any leftovers
                emit()
            pos_chunks.clear()
            edge_phase(*RELS[1])   # cflow (reads spair_com)
            edge_phase(*RELS[2])   # supply (reads spair_pos)
            store_out(0, "com")    # acc_com final after supply
            edge_phase(*RELS[3])   # pflow (reads spair_pos)
            store_out(1, "pos")
    nc.compile()
    return nc


LAST_RESULT = None


def kernel(**inputs):
    global LAST_RESULT
    from concourse.bass_utils import run_bass_kernel_spmd

    com_emb = np.asarray(inputs["com_emb"], np.float32)
    pos_emb = np.asarray(inputs["pos_emb"], np.float32)

    shapes, percore_rel = {}, {}
    for rname, s_t, d_t in RELS:
        per_core, blocks_grid, deg_out_cols, P_out, P_in = _prep_relation(
            inputs[f"{rname}_src"], inputs[f"{rname}_dst"], inputs[f"{rname}_w"])
        shapes[rname] = {"blocks_grid": blocks_grid, "P_out": P_out, "P_in": P_in}
        percore_rel[rname] = (per_core, deg_out_cols)

    nc = _build_kernel(shapes)

    in_maps = []
    for k in range(NCORES):
        m = {"com_emb": com_emb, "pos_emb": pos_emb}
        for rname, s_t, d_t in RELS:
            per_core, deg_out_cols = percore_rel[rname]
            pc = per_core[k]
            m[f"{rname}_gidx"] = pc["gidx"]
            m[f"{rname}_wcol"] = pc["wcol"]
            m[f"{rname}_dcol"] = pc["dcol"]
            m[f"{rname}_degout"] = deg_out_cols
            m[f"{rname}_degin"] = pc["deg_in"]
            m[f"W_{rname}"] = np.asarray(inputs[f"W_{rname}"], np.float32)
            m[f"b_{rname}"] = np.asarray(inputs[f"b_{rname}"], np.float32).reshape(1, D)
        in_maps.append(m)

    res = run_bass_kernel_spmd(nc, in_maps, core_ids=list(range(NCORES)))
    LAST_RESULT = res
    out = np.empty((2, N_COM, D), np.float32)
    for k in range(NCORES):
        o = res.results[k]["out"]
        out[0, k * SLICE:(k + 1) * SLICE] = o[0]
        out[1, k * SLICE:(k + 1) * SLICE] = o[1]
    return out
